# revision 1
# baseline (speedup 1.0000x reference)
"""LyraGemma3 sliding-window attention — Trainium2 Bass kernel, 8 NeuronCores.

Sharding: core = b*4 + h  (b in {0,1} batch, h in {0..3} head-group).
Each core owns vanilla head h, lyra head 4+h, kv head h for batch b and
produces output rows [512h, 512h+512) of batch b (the reference's
concat/transpose/reshape scramble makes those rows depend only on head h of
both streams), so the 8 cores produce disjoint slices of the final output —
no collectives.

Matmuls run as float32r (full PE rate at moving-dim >= 256, ~1.5e-4 rel err
per 128-dot vs fp32). RMS-norm's (1+w) scaling is folded into the projection
weights on the host; the rms denominator uses (1+w)^-2-weighted
sum-of-squares via a PE ones-matmul. Softmax runs without max-subtraction
(scores*scale is O(6), exp cannot overflow fp32).
"""

import sys

sys.path.insert(0, "/opt/trn_rl_repo")

import numpy as np

import concourse.bass as bass
import concourse.tile as tile
from concourse import mybir
from concourse.tile import ScopedClock

F32 = mybir.dt.float32
F32R = mybir.dt.float32r
AF = mybir.ActivationFunctionType

B, S, HID = 2, 2048, 2560
H, KV, D = 8, 4, 256
WINDOW = 1024
THETA = 10000.0
EPS = 1e-6
SCALING = 256.0 ** (-0.5)  # 1/16

NKC = HID // 128  # 20 contraction chunks for projections
NST = 8           # phase-A s-tiles of 256 tokens
NT = S // 128     # 16 key tiles of 128
NQ = 4            # attention q-tiles of 512
MASK_NEG = -1e30


class SplitWaitTC(tile.TileContext):
    """This container's walrus encodes at most ONE semaphore wait per
    instruction; Tile emits multi-wait sync_info. Hoist extra waits onto
    preceding same-engine NOPs."""

    def _drain_and_barrier(self, tick_clock, wait_clock):
        nc = self.nc
        drain_inst = nc.sync.drain()
        wait_clock.add_sem_waits(
            drain_inst.ins, ScopedClock({None: tick_clock.global_clock})
        )
        self._split_multi_waits()
        nc.all_engine_barrier()
        popped = nc._tile_sem_poison_stack.pop()
        assert popped is self._sem_poison
        nc.clear_and_free_semaphores(list(self.sems.allocated().values()))
        nc.all_engine_barrier()

    def _split_multi_waits(self):
        nc = self.nc
        cur_bb = nc.cur_bb
        assert cur_bb is not None
        for f in nc.m.functions:
            for blk in f.blocks:
                insts = blk.instructions
                i = 0
                while i < len(insts):
                    inst = insts[i]
                    si = inst.sync_info
                    if si is not None and si.on_wait and len(si.on_wait) > 1:
                        waits = list(si.on_wait)
                        inst.sync_info = mybir.SyncInfo(
                            on_wait=waits[-1:], on_update=si.on_update
                        )
                        eng = inst.engine
                        for w in waits[:-1]:
                            nop = nc.engines[eng].nop()
                            nop.ins.sync_info = mybir.SyncInfo(
                                on_wait=[w], on_update=[]
                            )
                            cur_bb.bb.instructions.remove(nop.ins)
                            insts.insert(i, nop.ins)
                            i += 1
                    i += 1


def _mask_index(T, Q):
    """Mask tile for key-tile T against q-tile Q (queries [512Q,512Q+512)).
    Returns None (fully valid), 4+j (causal), or j'' (window edge)."""
    j = T - 4 * Q
    if j >= 0:
        return 4 + j
    if T >= 4 * Q - 4:
        return None
    return T - (4 * Q - 8)


def build_program():
    nc = bass.Bass()

    hsT = nc.declare_dram_parameter("hsT", [HID, S], F32R, isOutput=False)
    wq2 = nc.declare_dram_parameter("wq2", [HID, 512], F32R, isOutput=False)
    wk1 = nc.declare_dram_parameter("wk1", [HID, 256], F32R, isOutput=False)
    wv1 = nc.declare_dram_parameter("wv1", [HID, 256], F32R, isOutput=False)
    wo_d = nc.declare_dram_parameter("wo", [H * D, HID], F32R, isOutput=False)
    cos_d = nc.declare_dram_parameter("cos_t", [128, S], F32, isOutput=False)
    sin_d = nc.declare_dram_parameter("sin_t", [128, S], F32, isOutput=False)
    masks_d = nc.declare_dram_parameter("masks", [8 * 128, 512], F32, isOutput=False)
    invq_d = nc.declare_dram_parameter("invq", [128, 2], F32R, isOutput=False)
    invk_d = nc.declare_dram_parameter("invk", [128, 2], F32R, isOutput=False)
    onec_d = nc.declare_dram_parameter("onec", [128, 1], F32R, isOutput=False)
    oner_d = nc.declare_dram_parameter("oner", [1, 128], F32R, isOutput=False)
    epsb_d = nc.declare_dram_parameter("epsb", [1, 1], F32, isOutput=False)
    out_d = nc.declare_dram_parameter("out", [512, HID], F32, isOutput=True)

    # DRAM spill for phase-A results (re-loaded in phase C)
    qT_sp = nc.dram_tensor("qT_sp", [4, 128, S], F32R)
    kTn_sp = nc.dram_tensor("kTn_sp", [2, 128, S], F32R)
    kTr_sp = nc.dram_tensor("kTr_sp", [2, 128, S], F32R)
    v_sp = nc.dram_tensor("v_sp", [128, NT * 256], F32R)

    with SplitWaitTC(nc) as tc:
        with (
            tc.tile_pool(name="outer", bufs=1) as pO,
            tc.tile_pool(name="outerps", bufs=1, space="PSUM") as _psO,
        ):
            onec = pO.tile([128, 1], F32R, name="onec")
            nc.sync.dma_start(onec[:], onec_d[:])
            oner = pO.tile([1, 128], F32R, name="oner")
            nc.sync.dma_start(oner[:], oner_d[:])
            epsb = pO.tile([1, 1], F32, name="epsb")
            nc.sync.dma_start(epsb[:], epsb_d[:])
            # ================= PHASE A: projections + norm + rope ========
            with (
                tc.tile_pool(name="pAw", bufs=1) as pW,
                tc.tile_pool(name="pA", bufs=1) as pA,
                tc.tile_pool(name="pAps", bufs=1, space="PSUM") as psA,
            ):
                wq_sb = pW.tile([128, NKC * 512], F32R, name="wq_sb")
                nc.sync.dma_start(
                    wq_sb[:].rearrange("p (c d) -> p c d", c=NKC),
                    wq2[:].rearrange("(c p) d -> c p d", p=128).transpose([1, 0, 2]),
                )
                wk_sb = pW.tile([128, NKC * 256], F32R, name="wk_sb")
                nc.sync.dma_start(
                    wk_sb[:].rearrange("p (c d) -> p c d", c=NKC),
                    wk1[:].rearrange("(c p) d -> c p d", p=128).transpose([1, 0, 2]),
                )
                wv_sb = pW.tile([128, NKC * 256], F32R, name="wv_sb")
                nc.sync.dma_start(
                    wv_sb[:].rearrange("p (c d) -> p c d", c=NKC),
                    wv1[:].rearrange("(c p) d -> c p d", p=128).transpose([1, 0, 2]),
                )
                cos_sb = pW.tile([128, S], F32, name="cos_sb")
                nc.sync.dma_start(cos_sb[:], cos_d[:])
                sin_sb = pW.tile([128, S], F32, name="sin_sb")
                nc.sync.dma_start(sin_sb[:], sin_d[:])
                invq = pW.tile([128, 2], F32R, name="invq")
                nc.sync.dma_start(invq[:], invq_d[:])
                invk = pW.tile([128, 2], F32R, name="invk")
                nc.sync.dma_start(invk[:], invk_d[:])

                for st in range(NST):
                    s0 = st * 256
                    hst = pA.tile([128, NKC * 256], F32R, name="hst", bufs=2)
                    nc.sync.dma_start(
                        hst[:].rearrange("p (c s) -> p c s", c=NKC),
                        hsT[:, s0 : s0 + 256]
                        .rearrange("(c p) s -> c p s", p=128)
                        .transpose([1, 0, 2]),
                    )
                    # ---- projections (accumulate over 20 HID chunks) ----
                    qz = pA.tile([128, 1024], F32, name="qz", bufs=2)
                    for hc in range(4):
                        pq = psA.tile([128, 256], F32, name="pacc", bufs=3)
                        for kc in range(NKC):
                            nc.tensor.matmul(
                                pq[:],
                                wq_sb[:, kc * 512 + hc * 128 : kc * 512 + (hc + 1) * 128],
                                hst[:, kc * 256 : (kc + 1) * 256],
                                start=(kc == 0),
                                stop=(kc == NKC - 1),
                            )
                        nc.vector.tensor_copy(qz[:, hc * 256 : (hc + 1) * 256], pq[:])
                    kz = pA.tile([128, 512], F32, name="kz", bufs=2)
                    for hc in range(2):
                        pk = psA.tile([128, 256], F32, name="pacc", bufs=3)
                        for kc in range(NKC):
                            nc.tensor.matmul(
                                pk[:],
                                wk_sb[:, kc * 256 + hc * 128 : kc * 256 + (hc + 1) * 128],
                                hst[:, kc * 256 : (kc + 1) * 256],
                                start=(kc == 0),
                                stop=(kc == NKC - 1),
                            )
                        nc.vector.tensor_copy(kz[:, hc * 256 : (hc + 1) * 256], pk[:])
                    vst = pA.tile([128, 512], F32R, name="vst", bufs=2)
                    for sm in range(2):
                        pv = psA.tile([128, 256], F32, name="pacc", bufs=3)
                        for kc in range(NKC):
                            nc.tensor.matmul(
                                pv[:],
                                hst[:, kc * 256 + sm * 128 : kc * 256 + sm * 128 + 128],
                                wv_sb[:, kc * 256 : (kc + 1) * 256],
                                start=(kc == 0),
                                stop=(kc == NKC - 1),
                            )
                        nc.vector.tensor_copy(vst[:, sm * 256 : (sm + 1) * 256], pv[:])
                    nc.sync.dma_start(v_sp[:, st * 512 : (st + 1) * 512], vst[:])

                    # ---- rms-norm factors (3 heads: qv, ql, k) ----
                    sqq = pA.tile([128, 1024], F32R, name="sqq", bufs=2)
                    nc.scalar.activation(sqq[:], qz[:], AF.Square)
                    sqk = pA.tile([128, 512], F32R, name="sqk", bufs=2)
                    nc.scalar.activation(sqk[:], kz[:], AF.Square)
                    bcs = []
                    for head in range(3):  # 0: q vanilla, 1: q lyra, 2: k
                        pn = psA.tile([1, 256], F32, name="pn", bufs=2)
                        for c in range(2):
                            if head < 2:
                                rhs = sqq[:, (head * 2 + c) * 256 : (head * 2 + c + 1) * 256]
                                lhsT = invq[:, c : c + 1]
                            else:
                                rhs = sqk[:, c * 256 : (c + 1) * 256]
                                lhsT = invk[:, c : c + 1]
                            nc.tensor.matmul(
                                pn[:], lhsT, rhs, start=(c == 0), stop=(c == 1)
                            )
                        srt = pA.tile([1, 256], F32, name="srt", bufs=2)
                        nc.scalar.activation(
                            srt[:], pn[:], AF.Sqrt, bias=epsb[:], scale=1.0 / 256.0
                        )
                        rst = pA.tile([1, 256], F32R, name="rst", bufs=2)
                        with nc.allow_low_precision(reason="rms rstd"):
                            nc.vector.reciprocal(rst[:], srt[:])
                        pbc = psA.tile([128, 256], F32, name="pbc", bufs=2)
                        nc.tensor.matmul(pbc[:], oner[:], rst[:], start=True, stop=True)
                        bc = pA.tile([128, 256], F32, name=f"bc{head}", bufs=2)
                        nc.vector.tensor_copy(bc[:], pbc[:])
                        bcs.append(bc)

                    # ---- rope + apply rstd ----
                    cs = cos_sb[:, s0 : s0 + 256]
                    sn = sin_sb[:, s0 : s0 + 256]

                    def rope2(z0, z1, bc, d0, d1):
                        t0 = pA.tile([128, 256], F32, name="t0", bufs=2)
                        nc.vector.tensor_mul(t0[:], z0, cs)
                        t1 = pA.tile([128, 256], F32, name="t1", bufs=2)
                        nc.vector.tensor_mul(t1[:], z1, sn)
                        u0 = pA.tile([128, 256], F32, name="u0", bufs=2)
                        nc.vector.tensor_sub(u0[:], t0[:], t1[:])
                        nc.vector.tensor_mul(d0, u0[:], bc[:])
                        t2 = pA.tile([128, 256], F32, name="t2", bufs=2)
                        nc.vector.tensor_mul(t2[:], z1, cs)
                        t3 = pA.tile([128, 256], F32, name="t3", bufs=2)
                        nc.vector.tensor_mul(t3[:], z0, sn)
                        u1 = pA.tile([128, 256], F32, name="u1", bufs=2)
                        nc.vector.tensor_add(u1[:], t2[:], t3[:])
                        nc.vector.tensor_mul(d1, u1[:], bc[:])

                    qro = pA.tile([128, 1024], F32R, name="qro", bufs=2)
                    for head in range(2):
                        rope2(
                            qz[:, (head * 2) * 256 : (head * 2) * 256 + 256],
                            qz[:, (head * 2 + 1) * 256 : (head * 2 + 1) * 256 + 256],
                            bcs[head],
                            qro[:, (head * 2) * 256 : (head * 2) * 256 + 256],
                            qro[:, (head * 2 + 1) * 256 : (head * 2 + 1) * 256 + 256],
                        )
                    krst = pA.tile([128, 512], F32R, name="krst", bufs=2)
                    rope2(
                        kz[:, 0:256], kz[:, 256:512], bcs[2],
                        krst[:, 0:256], krst[:, 256:512],
                    )
                    knst = pA.tile([128, 512], F32R, name="knst", bufs=2)
                    nc.vector.tensor_mul(knst[:, 0:256], kz[:, 0:256], bcs[2][:])
                    nc.vector.tensor_mul(knst[:, 256:512], kz[:, 256:512], bcs[2][:])

                    nc.sync.dma_start(
                        qT_sp[:, :, s0 : s0 + 256].transpose([1, 0, 2]),
                        qro[:].rearrange("p (c s) -> p c s", c=4),
                    )
                    nc.sync.dma_start(
                        kTr_sp[:, :, s0 : s0 + 256].transpose([1, 0, 2]),
                        krst[:].rearrange("p (c s) -> p c s", c=2),
                    )
                    nc.sync.dma_start(
                        kTn_sp[:, :, s0 : s0 + 256].transpose([1, 0, 2]),
                        knst[:].rearrange("p (c s) -> p c s", c=2),
                    )

            # ================= PHASES C+D ================================
            with tc.tile_pool(name="pOC", bufs=1) as pOC:
              # normalized attention outputs in combinedT (wo-lhsT) layout:
              # outC[stream][dc][:, j*256 + m] = outT[stream][dc][d, 8m+j]
              outC = [
                  [pOC.tile([128, S], F32R, name=f"outC{s}{c}") for c in range(2)]
                  for s in range(2)
              ]
              # ================= PHASE C: attention ========================
              with (
                tc.tile_pool(name="pCk", bufs=1) as pK,
                  tc.tile_pool(name="pC", bufs=1) as pC,
                  tc.tile_pool(name="pCps", bufs=1, space="PSUM") as psC,
              ):
                  masks_sb = pK.tile([128, 8 * 512], F32, name="masks_sb")
                  nc.sync.dma_start(
                      masks_sb[:].rearrange("p (m s) -> p m s", m=8),
                      masks_d[:].rearrange("(m p) s -> m p s", p=128).transpose([1, 0, 2]),
                  )
                  kTr_all = pK.tile([128, 2 * S], F32R, name="kTr_all")
                  nc.sync.dma_start(
                      kTr_all[:].rearrange("p (c s) -> p c s", c=2),
                      kTr_sp[:, :, :].transpose([1, 0, 2]),
                  )
                  kTn_all = pK.tile([128, 2 * S], F32R, name="kTn_all")
                  nc.sync.dma_start(
                      kTn_all[:].rearrange("p (c s) -> p c s", c=2),
                      kTn_sp[:, :, :].transpose([1, 0, 2]),
                  )
                  v_all = pK.tile([128, NT * 256], F32R, name="v_all")
                  nc.sync.dma_start(v_all[:], v_sp[:, :])

                  for stream in range(2):  # 0 = vanilla (roped k), 1 = lyra
                      kT = kTr_all if stream == 0 else kTn_all
                      for Q in range(NQ):
                          qTq = pC.tile([128, 1024], F32R, name="qTq", bufs=2)
                          nc.sync.dma_start(
                              qTq[:].rearrange("p (c s) -> p c s", c=2),
                              qT_sp[
                                  2 * stream : 2 * stream + 2, :, Q * 512 : (Q + 1) * 512
                              ].transpose([1, 0, 2]),
                          )
                          T_lo = max(0, 4 * Q - 8)
                          T_hi = 4 * Q + 3
                          po0 = psC.tile([128, 512], F32, name="po0", bufs=1)
                          po1 = psC.tile([128, 512], F32, name="po1", bufs=1)
                          psm = psC.tile([1, 512], F32, name="psm", bufs=1)
                          for T in range(T_lo, T_hi + 1):
                              pss = psC.tile([128, 512], F32, name="pss", bufs=2)
                              nc.tensor.matmul(
                                  pss[:],
                                  kT[:, T * 128 : (T + 1) * 128],
                                  qTq[:, 0:512],
                                  start=True,
                                  stop=False,
                              )
                              nc.tensor.matmul(
                                  pss[:],
                                  kT[:, S + T * 128 : S + (T + 1) * 128],
                                  qTq[:, 512:1024],
                                  start=False,
                                  stop=True,
                              )
                              midx = _mask_index(T, Q)
                              probs = pC.tile([128, 512], F32R, name="probs", bufs=3)
                              if midx is None:
                                  nc.scalar.activation(
                                      probs[:], pss[:], AF.Exp, scale=SCALING
                                  )
                              else:
                                  sct = pC.tile([128, 512], F32, name="sct", bufs=2)
                                  nc.vector.tensor_add(
                                      sct[:],
                                      pss[:],
                                      masks_sb[:, midx * 512 : (midx + 1) * 512],
                                  )
                                  nc.scalar.activation(
                                      probs[:], sct[:], AF.Exp, scale=SCALING
                                  )
                              first = T == T_lo
                              last = T == T_hi
                              nc.tensor.matmul(
                                  psm[:], onec[:], probs[:], start=first, stop=last
                              )
                              nc.tensor.matmul(
                                  po0[:],
                                  v_all[:, T * 256 : T * 256 + 128],
                                  probs[:],
                                  start=first,
                                  stop=last,
                              )
                              nc.tensor.matmul(
                                  po1[:],
                                  v_all[:, T * 256 + 128 : T * 256 + 256],
                                  probs[:],
                                  start=first,
                                  stop=last,
                              )
                          rstC = pC.tile([1, 512], F32R, name="rstC", bufs=2)
                          with nc.allow_low_precision(reason="softmax denom"):
                              nc.vector.reciprocal(rstC[:], psm[:])
                          pbcC = psC.tile([128, 512], F32, name="pbcC", bufs=1)
                          nc.tensor.matmul(pbcC[:], oner[:], rstC[:], start=True, stop=True)
                          bcsC = pC.tile([128, 512], F32, name="bcsC", bufs=2)
                          nc.vector.tensor_copy(bcsC[:], pbcC[:])
                          for dc in range(2):
                              po = po0 if dc == 0 else po1
                              in_ap = po[:].rearrange("p (m j) -> p m j", j=8)
                              bc_ap = bcsC[:].rearrange("p (m j) -> p m j", j=8)
                              out_ap = (
                                  outC[stream][dc][:]
                                  .rearrange("p (j m) -> p j m", j=8)
                                  .transpose([0, 2, 1])[:, Q * 64 : (Q + 1) * 64, :]
                              )
                              nc.vector.tensor_mul(out_ap, in_ap, bc_ap)

              # ================= PHASE D: output projection ================
              with (
                  tc.tile_pool(name="pD", bufs=1) as pD,
                  tc.tile_pool(name="pDps", bufs=1, space="PSUM") as psD,
              ):
                  for co, w in ((0, 1024), (1024, 1024), (2048, 512)):
                      ntiles = w // 512
                      pos = [
                          [
                              psD.tile([128, 512], F32, name=f"pD{m}{nt}", bufs=1)
                              for nt in range(ntiles)
                          ]
                          for m in range(4)
                      ]
                      for kc in range(16):
                          wosb = pD.tile([128, 1024], F32R, name="wosb", bufs=3)
                          nc.sync.dma_start(
                              wosb[:, 0:w], wo_d[kc * 128 : (kc + 1) * 128, co : co + w]
                          )
                          j, dc = kc // 2, kc % 2
                          for m in range(4):
                              stream, m0 = m // 2, (m % 2) * 128
                              lhsT = outC[stream][dc][:, j * 256 + m0 : j * 256 + m0 + 128]
                              for nt in range(ntiles):
                                  nc.tensor.matmul(
                                      pos[m][nt][:],
                                      lhsT,
                                      wosb[:, nt * 512 : (nt + 1) * 512],
                                      start=(kc == 0),
                                      stop=(kc == 15),
                                  )
                      for m in range(4):
                          for nt in range(ntiles):
                              ost = pD.tile([128, 512], F32, name="ost", bufs=3)
                              nc.vector.tensor_copy(ost[:], pos[m][nt][:])
                              nc.sync.dma_start(
                                  out_d[
                                      m * 128 : (m + 1) * 128,
                                      co + nt * 512 : co + (nt + 1) * 512,
                                  ],
                                  ost[:],
                              )
    return nc


def _host_inputs(hidden_states, wq, wk, wv, wo, q_norm_w, k_norm_w):
    """Build the 8 per-core input maps (all host-side numpy prep)."""
    hs = np.asarray(hidden_states, dtype=np.float32)
    wq = np.asarray(wq, dtype=np.float32)
    wk = np.asarray(wk, dtype=np.float32)
    wv = np.asarray(wv, dtype=np.float32)
    wo = np.ascontiguousarray(np.asarray(wo, dtype=np.float32))
    qnw = np.asarray(q_norm_w, dtype=np.float32)
    knw = np.asarray(k_norm_w, dtype=np.float32)

    hsT = [np.ascontiguousarray(hs[b].T) for b in range(B)]

    inv_freq = 1.0 / (THETA ** (np.arange(0, D, 2, dtype=np.float32) / D))
    ang = np.outer(inv_freq, np.arange(S, dtype=np.float32))  # (128, S)
    cos_t = np.ascontiguousarray(np.cos(ang), dtype=np.float32)
    sin_t = np.ascontiguousarray(np.sin(ang), dtype=np.float32)

    x = np.arange(128)[:, None]
    y = np.arange(512)[None, :]
    masks = np.empty((8, 128, 512), np.float32)
    for jj in range(4):  # window-edge: valid iff y < x + 128*jj
        masks[jj] = np.where(y < x + 128 * jj, 0.0, MASK_NEG)
    for j in range(4):  # causal: valid iff y >= x + 128*j
        masks[4 + j] = np.where(y >= x + 128 * j, 0.0, MASK_NEG)
    masks = np.ascontiguousarray(masks.reshape(8 * 128, 512))

    invq = np.ascontiguousarray(
        ((1.0 + qnw) ** -2).reshape(2, 128).T, dtype=np.float32
    )
    invk = np.ascontiguousarray(
        ((1.0 + knw) ** -2).reshape(2, 128).T, dtype=np.float32
    )
    onec = np.ones((128, 1), np.float32)
    oner = np.ones((1, 128), np.float32)

    qs = 1.0 + qnw
    ks = 1.0 + knw
    in_maps = []
    for core in range(8):
        b, h = core // 4, core % 4
        wq2 = np.concatenate(
            [
                wq[:, h * D : (h + 1) * D] * qs[None, :],
                wq[:, (4 + h) * D : (5 + h) * D] * qs[None, :],
            ],
            axis=1,
        )
        in_maps.append(
            {
                "hsT": hsT[b],
                "wq2": np.ascontiguousarray(wq2),
                "wk1": np.ascontiguousarray(wk[:, h * D : (h + 1) * D] * ks[None, :]),
                "wv1": np.ascontiguousarray(wv[:, h * D : (h + 1) * D]),
                "wo": wo,
                "cos_t": cos_t,
                "sin_t": sin_t,
                "masks": masks,
                "invq": invq,
                "invk": invk,
                "onec": onec,
                "epsb": np.full((1, 1), EPS, np.float32),
                "oner": oner,
            }
        )
    return in_maps


_PROGRAM = None


def kernel(hidden_states, wq, wk, wv, wo, q_norm_w, k_norm_w):
    global _PROGRAM
    from concourse.bass_utils import run_bass_kernel_spmd

    if _PROGRAM is None:
        _PROGRAM = build_program()
    in_maps = _host_inputs(hidden_states, wq, wk, wv, wo, q_norm_w, k_norm_w)
    res = run_bass_kernel_spmd(_PROGRAM, in_maps, core_ids=list(range(8)))
    out = np.empty((B, S, HID), np.float32)
    for core in range(8):
        b, h = core // 4, core % 4
        out[b, h * 512 : (h + 1) * 512, :] = res.results[core]["out"]
    return out



# revision 9
# speedup vs baseline: 1.2343x; 1.2343x over previous
"""LyraGemma3 sliding-window attention — Trainium2 Bass kernel, 8 NeuronCores.

Sharding: core = b*4 + h  (b in {0,1} batch, h in {0..3} head-group).
Each core owns vanilla head h, lyra head 4+h, kv head h for batch b and
produces output rows [512h, 512h+512) of batch b — disjoint slices, no
collectives.

v2 design (vs the f32r/DRAM-spill baseline):
- All matmul operands bf16 (1 cycle/row, half the DMA bytes, half SBUF).
- q/k/v and outC stay SBUF-resident between phases (no DRAM spill, no
  phase barrier: single program order A -> C -> D keeps PE dense).
- Host prepacks every DRAM tensor into its SBUF layout so all DMAs are
  contiguous (no transposing descriptors).
- Sliding-window/causal masks applied multiplicatively after exp via
  GpSimd affine_select (zero-fill), removing the DVE mask adds and the
  mask tiles entirely.
- PSUM->SBUF copies and rms-norm rsqrt run on the Scalar (ACT) engine;
  rstd/softmax-denominator broadcasts run on GpSimd partition_broadcast,
  keeping DVE for rope only.
- wo streams into SBUF during phase C (address space reused from the
  phase-A weights), so phase D is pure PE work.
"""

import sys

sys.path.insert(0, "/opt/trn_rl_repo")

import numpy as np
import ml_dtypes

import concourse.bass as bass
import concourse.tile as tile
from concourse import mybir
from concourse.tile import ScopedClock

F32 = mybir.dt.float32
BF16 = mybir.dt.bfloat16
AF = mybir.ActivationFunctionType
ALU = mybir.AluOpType
NPBF = ml_dtypes.bfloat16

B, S, HID = 2, 2048, 2560
H, KV, D = 8, 4, 256
WINDOW = 1024
THETA = 10000.0
EPS = 1e-6
SCALING = 256.0 ** (-0.5)  # 1/16

NKC = HID // 128  # 20 contraction chunks for projections
NST = 8           # phase-A s-tiles of 256 tokens
NT = S // 128     # 16 key tiles of 128
NQ = 4            # attention q-tiles of 512


class SplitWaitTC(tile.TileContext):
    """This container's walrus encodes at most ONE semaphore wait per
    instruction; Tile emits multi-wait sync_info. Hoist extra waits onto
    preceding same-engine NOPs."""

    def _drain_and_barrier(self, tick_clock, wait_clock):
        nc = self.nc
        drain_inst = nc.sync.drain()
        wait_clock.add_sem_waits(
            drain_inst.ins, ScopedClock({None: tick_clock.global_clock})
        )
        self._split_multi_waits()
        nc.all_engine_barrier()
        popped = nc._tile_sem_poison_stack.pop()
        assert popped is self._sem_poison
        nc.clear_and_free_semaphores(list(self.sems.allocated().values()))
        nc.all_engine_barrier()

    def _split_multi_waits(self):
        nc = self.nc
        cur_bb = nc.cur_bb
        assert cur_bb is not None
        for f in nc.m.functions:
            for blk in f.blocks:
                insts = blk.instructions
                i = 0
                while i < len(insts):
                    inst = insts[i]
                    si = inst.sync_info
                    if si is not None and si.on_wait and len(si.on_wait) > 1:
                        waits = list(si.on_wait)
                        inst.sync_info = mybir.SyncInfo(
                            on_wait=waits[-1:], on_update=si.on_update
                        )
                        eng = inst.engine
                        for w in waits[:-1]:
                            nop = nc.engines[eng].nop()
                            nop.ins.sync_info = mybir.SyncInfo(
                                on_wait=[w], on_update=[]
                            )
                            cur_bb.bb.instructions.remove(nop.ins)
                            insts.insert(i, nop.ins)
                            i += 1
                    i += 1


def _mask_kind(T, Q):
    """Classify key-tile T against q-tile Q (queries [512Q, 512Q+512)).
    Returns None (fully valid), ('causal', ...) or ('window', ...)."""
    if T >= 4 * Q:
        return "causal"
    if T >= 4 * Q - 4:
        return None
    return "window"


def build_program():
    nc = bass.Bass()

    hsp = nc.declare_dram_parameter("hsp", [128, NST * NKC * 256], BF16, isOutput=False)
    wqp = nc.declare_dram_parameter("wqp", [4, 128, NKC * 128], BF16, isOutput=False)
    wkp = nc.declare_dram_parameter("wkp", [2, 128, NKC * 128], BF16, isOutput=False)
    wvp = nc.declare_dram_parameter("wvp", [128, NKC * 256], BF16, isOutput=False)
    wop = nc.declare_dram_parameter("wop", [128, 16 * HID], BF16, isOutput=False)
    cosp = nc.declare_dram_parameter("cosp", [128, S], BF16, isOutput=False)
    sinp = nc.declare_dram_parameter("sinp", [128, S], BF16, isOutput=False)
    invq_d = nc.declare_dram_parameter("invq", [128, 2], BF16, isOutput=False)
    invk_d = nc.declare_dram_parameter("invk", [128, 2], BF16, isOutput=False)
    onec_d = nc.declare_dram_parameter("onec", [128, 1], BF16, isOutput=False)
    oner_d = nc.declare_dram_parameter("oner", [1, 128], BF16, isOutput=False)
    epsb_d = nc.declare_dram_parameter("epsb", [1, 1], F32, isOutput=False)
    out_d = nc.declare_dram_parameter("out", [512, HID], F32, isOutput=True)

    with SplitWaitTC(nc) as tc:
        with tc.tile_pool(name="outer", bufs=1) as pO:
            onec = pO.tile([128, 1], BF16, name="onec")
            nc.sync.dma_start(onec[:], onec_d[:])
            oner = pO.tile([1, 128], BF16, name="oner")
            nc.sync.dma_start(oner[:], oner_d[:])
            epsb = pO.tile([1, 1], F32, name="epsb")
            nc.sync.dma_start(epsb[:], epsb_d[:])
            invq = pO.tile([128, 2], BF16, name="invq")
            nc.sync.dma_start(invq[:], invq_d[:])
            invk = pO.tile([128, 2], BF16, name="invk")
            nc.sync.dma_start(invk[:], invk_d[:])

            # persistent intermediates (bf16, SBUF-resident across phases)
            qT = pO.tile([128, 4 * S], BF16, name="qT")
            kTr = pO.tile([128, 2 * S], BF16, name="kTr")
            kTn = pO.tile([128, 2 * S], BF16, name="kTn")
            v_all = pO.tile([128, NT * 256], BF16, name="v_all")
            outC = [
                [pO.tile([128, S], BF16, name=f"outC{s}{c}") for c in range(2)]
                for s in range(2)
            ]

            zero_fill = nc.gpsimd.to_reg(0.0)

            # ================= PHASE A: projections + norm + rope ========
            with (
                tc.tile_pool(name="pA", bufs=1) as pA,
                tc.tile_pool(name="pAps", bufs=1, space="PSUM") as psA,
            ):
                wq_sb = [pA.tile([128, NKC * 128], BF16, name=f"wq{hc}") for hc in range(4)]
                nc.sync.dma_start(wq_sb[0][:], wqp[0])
                hst_t = {}

                def hst_dma(st):
                    t = pA.tile([128, NKC * 256], BF16, name="hst", bufs=2)
                    nc.sync.dma_start(t[:], hsp[:, st * NKC * 256 : (st + 1) * NKC * 256])
                    hst_t[st] = t

                hst_dma(0)
                for hc in range(1, 4):
                    nc.sync.dma_start(wq_sb[hc][:], wqp[hc])
                wk_sb = [pA.tile([128, NKC * 128], BF16, name=f"wk{hc}") for hc in range(2)]
                for hc in range(2):
                    nc.sync.dma_start(wk_sb[hc][:], wkp[hc])
                wv_sb = pA.tile([128, NKC * 256], BF16, name="wv_sb")
                nc.sync.dma_start(wv_sb[:], wvp[:])
                hst_dma(1)
                cos_sb = pA.tile([128, S], BF16, name="cos_sb")
                nc.sync.dma_start(cos_sb[:], cosp[:])
                sin_sb = pA.tile([128, S], BF16, name="sin_sb")
                nc.sync.dma_start(sin_sb[:], sinp[:])

                for st in range(NST):
                    s0 = st * 256
                    hst = hst_t[st]
                    if st + 2 < NST:
                        hst_dma(st + 2)
                    # ---- projections (accumulate over 20 HID chunks) ----
                    qz = pA.tile([128, 1024], BF16, name="qz", bufs=2)
                    for hc in range(4):
                        pq = psA.tile([128, 256], F32, name="pacc", bufs=3)
                        for kc in range(NKC):
                            nc.tensor.matmul(
                                pq[:],
                                wq_sb[hc][:, kc * 128 : (kc + 1) * 128],
                                hst[:, kc * 256 : (kc + 1) * 256],
                                start=(kc == 0),
                                stop=(kc == NKC - 1),
                            )
                        nc.scalar.copy(qz[:, hc * 256 : (hc + 1) * 256], pq[:])
                    kz = pA.tile([128, 512], BF16, name="kz", bufs=2)
                    for hc in range(2):
                        pk = psA.tile([128, 256], F32, name="pacc", bufs=3)
                        for kc in range(NKC):
                            nc.tensor.matmul(
                                pk[:],
                                wk_sb[hc][:, kc * 128 : (kc + 1) * 128],
                                hst[:, kc * 256 : (kc + 1) * 256],
                                start=(kc == 0),
                                stop=(kc == NKC - 1),
                            )
                        nc.scalar.copy(kz[:, hc * 256 : (hc + 1) * 256], pk[:])
                    for sm in range(2):
                        pv = psA.tile([128, 256], F32, name="pacc", bufs=3)
                        for kc in range(NKC):
                            nc.tensor.matmul(
                                pv[:],
                                hst[:, kc * 256 + sm * 128 : kc * 256 + sm * 128 + 128],
                                wv_sb[:, kc * 256 : (kc + 1) * 256],
                                start=(kc == 0),
                                stop=(kc == NKC - 1),
                            )
                        nc.scalar.copy(
                            v_all[:, (2 * st + sm) * 256 : (2 * st + sm + 1) * 256],
                            pv[:],
                        )

                    # ---- rms-norm factors (3 heads: qv, ql, k) ----
                    sqq = pA.tile([128, 1024], BF16, name="sqq", bufs=2)
                    nc.scalar.activation(sqq[:], qz[:], AF.Square)
                    sqk = pA.tile([128, 512], BF16, name="sqk", bufs=2)
                    nc.scalar.activation(sqk[:], kz[:], AF.Square)
                    bcs = []
                    for head in range(3):  # 0: q vanilla, 1: q lyra, 2: k
                        pn = psA.tile([1, 256], F32, name="pn", bufs=2)
                        for c in range(2):
                            if head < 2:
                                rhs = sqq[:, (head * 2 + c) * 256 : (head * 2 + c + 1) * 256]
                                lhsT = invq[:, c : c + 1]
                            else:
                                rhs = sqk[:, c * 256 : (c + 1) * 256]
                                lhsT = invk[:, c : c + 1]
                            nc.tensor.matmul(
                                pn[:], lhsT, rhs, start=(c == 0), stop=(c == 1)
                            )
                        srt = pA.tile([1, 256], F32, name="srt", bufs=2)
                        nc.scalar.activation(
                            srt[:], pn[:], AF.Sqrt, bias=epsb[:], scale=1.0 / 256.0
                        )
                        rst = pA.tile([1, 256], BF16, name="rst", bufs=2)
                        with nc.allow_low_precision(reason="rms rstd"):
                            nc.vector.reciprocal(rst[:], srt[:])
                        pbc = psA.tile([128, 256], F32, name="pbc", bufs=2)
                        nc.tensor.matmul(pbc[:], oner[:], rst[:], start=True, stop=True)
                        bc = pA.tile([128, 256], BF16, name=f"bc{head}", bufs=2)
                        nc.scalar.copy(bc[:], pbc[:])
                        bcs.append(bc)

                    # ---- rope + apply rstd (DVE, bf16) ----
                    cs = cos_sb[:, s0 : s0 + 256]
                    sn = sin_sb[:, s0 : s0 + 256]

                    def rope2(z0, z1, bc, d0, d1):
                        t0 = pA.tile([128, 256], BF16, name="t0", bufs=2)
                        nc.vector.tensor_mul(t0[:], z0, cs)
                        t1 = pA.tile([128, 256], BF16, name="t1", bufs=2)
                        nc.vector.tensor_mul(t1[:], z1, sn)
                        u0 = pA.tile([128, 256], BF16, name="u0", bufs=2)
                        nc.vector.tensor_sub(u0[:], t0[:], t1[:])
                        nc.vector.tensor_mul(d0, u0[:], bc[:])
                        t2 = pA.tile([128, 256], BF16, name="t2", bufs=2)
                        nc.vector.tensor_mul(t2[:], z1, cs)
                        t3 = pA.tile([128, 256], BF16, name="t3", bufs=2)
                        nc.vector.tensor_mul(t3[:], z0, sn)
                        u1 = pA.tile([128, 256], BF16, name="u1", bufs=2)
                        nc.vector.tensor_add(u1[:], t2[:], t3[:])
                        nc.vector.tensor_mul(d1, u1[:], bc[:])

                    for head in range(2):
                        rope2(
                            qz[:, (head * 2) * 256 : (head * 2) * 256 + 256],
                            qz[:, (head * 2 + 1) * 256 : (head * 2 + 1) * 256 + 256],
                            bcs[head],
                            qT[:, (head * 2) * S + s0 : (head * 2) * S + s0 + 256],
                            qT[:, (head * 2 + 1) * S + s0 : (head * 2 + 1) * S + s0 + 256],
                        )
                    rope2(
                        kz[:, 0:256], kz[:, 256:512], bcs[2],
                        kTr[:, s0 : s0 + 256], kTr[:, S + s0 : S + s0 + 256],
                    )
                    nc.vector.tensor_mul(
                        kTn[:, s0 : s0 + 256], kz[:, 0:256], bcs[2][:]
                    )
                    nc.vector.tensor_mul(
                        kTn[:, S + s0 : S + s0 + 256], kz[:, 256:512], bcs[2][:]
                    )

            # ================= PHASES C+D ================================
            # wo streams into the SBUF space the phase-A pool released.
            with tc.tile_pool(name="pWo", bufs=1) as pWo:
                wo_sb = pWo.tile([128, 16 * HID], BF16, name="wo_sb")
                for i in range(4):
                    nc.sync.dma_start(
                        wo_sb[:, i * 4 * HID : (i + 1) * 4 * HID],
                        wop[:, i * 4 * HID : (i + 1) * 4 * HID],
                    )

                # ================= PHASE C: attention ====================
                with (
                    tc.tile_pool(name="pC", bufs=1) as pC,
                    tc.tile_pool(name="pCps", bufs=1, space="PSUM") as psC,
                ):
                    for stream in range(2):  # 0 = vanilla (roped k), 1 = lyra
                        kT = kTr if stream == 0 else kTn
                        q0 = (2 * stream) * S
                        q1 = (2 * stream + 1) * S
                        for Q in range(NQ):
                            T_lo = max(0, 4 * Q - 8)
                            T_hi = 4 * Q + 3
                            po0 = psC.tile([128, 512], F32, name="po0", bufs=2)
                            po1 = psC.tile([128, 512], F32, name="po1", bufs=2)
                            psm = psC.tile([1, 512], F32, name="psm", bufs=1)
                            for T in range(T_lo, T_hi + 1):
                                pss = psC.tile([128, 512], F32, name="pss", bufs=2)
                                nc.tensor.matmul(
                                    pss[:],
                                    kT[:, T * 128 : (T + 1) * 128],
                                    qT[:, q0 + Q * 512 : q0 + (Q + 1) * 512],
                                    start=True,
                                    stop=False,
                                )
                                nc.tensor.matmul(
                                    pss[:],
                                    kT[:, S + T * 128 : S + (T + 1) * 128],
                                    qT[:, q1 + Q * 512 : q1 + (Q + 1) * 512],
                                    start=False,
                                    stop=True,
                                )
                                kind = _mask_kind(T, Q)
                                probs = pC.tile([128, 512], BF16, name="probs", bufs=3)
                                if kind is None:
                                    nc.scalar.activation(
                                        probs[:], pss[:], AF.Exp, scale=SCALING
                                    )
                                else:
                                    praw = pC.tile([128, 512], BF16, name="praw", bufs=2)
                                    nc.scalar.activation(
                                        praw[:], pss[:], AF.Exp, scale=SCALING
                                    )
                                    if kind == "causal":
                                        # keep where 512Q + f >= 128T + p
                                        nc.gpsimd.affine_select(
                                            probs[:], praw[:],
                                            pattern=[[1, 512]],
                                            compare_op=ALU.is_ge,
                                            fill=zero_fill,
                                            base=512 * Q - 128 * T,
                                            channel_multiplier=-1,
                                        )
                                    else:
                                        # keep where 128T + p > 512Q + f - 1024
                                        nc.gpsimd.affine_select(
                                            probs[:], praw[:],
                                            pattern=[[-1, 512]],
                                            compare_op=ALU.is_ge,
                                            fill=zero_fill,
                                            base=128 * T - 512 * Q + 1023,
                                            channel_multiplier=1,
                                        )
                                first = T == T_lo
                                last = T == T_hi
                                nc.tensor.matmul(
                                    psm[:], onec[:], probs[:], start=first, stop=last
                                )
                                nc.tensor.matmul(
                                    po0[:],
                                    v_all[:, T * 256 : T * 256 + 128],
                                    probs[:],
                                    start=first,
                                    stop=last,
                                )
                                nc.tensor.matmul(
                                    po1[:],
                                    v_all[:, T * 256 + 128 : T * 256 + 256],
                                    probs[:],
                                    start=first,
                                    stop=last,
                                )
                            rstC = pC.tile([1, 512], BF16, name="rstC", bufs=2)
                            with nc.allow_low_precision(reason="softmax denom"):
                                nc.vector.reciprocal(rstC[:], psm[:])
                            pbcC = psC.tile([128, 512], F32, name="pbcC", bufs=1)
                            nc.tensor.matmul(
                                pbcC[:], oner[:], rstC[:], start=True, stop=True
                            )
                            bcsC = pC.tile([128, 512], F32, name="bcsC", bufs=2)
                            nc.scalar.copy(bcsC[:], pbcC[:])
                            for dc in range(2):
                                po = po0 if dc == 0 else po1
                                in_ap = po[:].rearrange("p (m j) -> p m j", j=8)
                                bc_ap = bcsC[:].rearrange("p (m j) -> p m j", j=8)
                                out_ap = (
                                    outC[stream][dc][:]
                                    .rearrange("p (j m) -> p j m", j=8)
                                    .transpose([0, 2, 1])[:, Q * 64 : (Q + 1) * 64, :]
                                )
                                nc.vector.tensor_mul(out_ap, in_ap, bc_ap)

                # ================= PHASE D: output projection ============
                with (
                    tc.tile_pool(name="pD", bufs=1) as pD,
                    tc.tile_pool(name="pDps", bufs=1, space="PSUM") as psD,
                ):
                    for co, w in ((0, 1024), (1024, 1024), (2048, 512)):
                        ntiles = w // 512
                        pos = [
                            [
                                psD.tile([128, 512], F32, name=f"pD{m}{nt}", bufs=1)
                                for nt in range(ntiles)
                            ]
                            for m in range(4)
                        ]
                        for kc in range(16):
                            j, dc = kc // 2, kc % 2
                            for m in range(4):
                                stream, m0 = m // 2, (m % 2) * 128
                                lhsT = outC[stream][dc][:, j * 256 + m0 : j * 256 + m0 + 128]
                                for nt in range(ntiles):
                                    nc.tensor.matmul(
                                        pos[m][nt][:],
                                        lhsT,
                                        wo_sb[:, kc * HID + co + nt * 512 : kc * HID + co + (nt + 1) * 512],
                                        start=(kc == 0),
                                        stop=(kc == 15),
                                    )
                        for m in range(4):
                            for nt in range(ntiles):
                                ost = pD.tile([128, 512], F32, name="ost", bufs=3)
                                nc.scalar.copy(ost[:], pos[m][nt][:])
                                nc.sync.dma_start(
                                    out_d[
                                        m * 128 : (m + 1) * 128,
                                        co + nt * 512 : co + (nt + 1) * 512,
                                    ],
                                    ost[:],
                                )
    return nc


def _host_inputs(hidden_states, wq, wk, wv, wo, q_norm_w, k_norm_w):
    """Build the 8 per-core input maps (all host-side numpy prep).
    Every tensor is prepacked into its exact SBUF layout so device DMAs
    are plain contiguous copies."""
    hs = np.asarray(hidden_states, dtype=np.float32)
    wq = np.asarray(wq, dtype=np.float32)
    wk = np.asarray(wk, dtype=np.float32)
    wv = np.asarray(wv, dtype=np.float32)
    wo = np.asarray(wo, dtype=np.float32)
    qnw = np.asarray(q_norm_w, dtype=np.float32)
    knw = np.asarray(k_norm_w, dtype=np.float32)

    def pack_w(w):  # [HID, width] -> [128, NKC*width] chunk-major free axis
        width = w.shape[1]
        return np.ascontiguousarray(
            w.reshape(NKC, 128, width).transpose(1, 0, 2).reshape(128, NKC * width)
        ).astype(NPBF)

    # hsT packed per s-tile: [128, (st, kc, 256)]
    hsp = []
    for b in range(B):
        h = hs[b].T.reshape(NKC, 128, NST, 256).transpose(1, 2, 0, 3)
        hsp.append(np.ascontiguousarray(h.reshape(128, NST * NKC * 256)).astype(NPBF))

    inv_freq = 1.0 / (THETA ** (np.arange(0, D, 2, dtype=np.float32) / D))
    ang = np.outer(inv_freq, np.arange(S, dtype=np.float32))  # (128, S)
    cosp = np.ascontiguousarray(np.cos(ang)).astype(NPBF)
    sinp = np.ascontiguousarray(np.sin(ang)).astype(NPBF)

    invq = np.ascontiguousarray(((1.0 + qnw) ** -2).reshape(2, 128).T).astype(NPBF)
    invk = np.ascontiguousarray(((1.0 + knw) ** -2).reshape(2, 128).T).astype(NPBF)
    onec = np.ones((128, 1), NPBF)
    oner = np.ones((1, 128), NPBF)
    epsb = np.full((1, 1), EPS, np.float32)

    # wo packed: [128, (kc, HID)]
    wop = np.ascontiguousarray(
        wo.reshape(16, 128, HID).transpose(1, 0, 2).reshape(128, 16 * HID)
    ).astype(NPBF)

    qs = 1.0 + qnw
    ks = 1.0 + knw
    in_maps = []
    for core in range(8):
        b, h = core // 4, core % 4
        wq2 = np.concatenate(
            [
                wq[:, h * D : (h + 1) * D] * qs[None, :],
                wq[:, (4 + h) * D : (5 + h) * D] * qs[None, :],
            ],
            axis=1,
        )  # [HID, 512]
        wqp = np.stack([pack_w(wq2[:, hc * 128 : (hc + 1) * 128]) for hc in range(4)])
        wk1 = wk[:, h * D : (h + 1) * D] * ks[None, :]
        wkp = np.stack([pack_w(wk1[:, hc * 128 : (hc + 1) * 128]) for hc in range(2)])
        wvp = pack_w(wv[:, h * D : (h + 1) * D])
        in_maps.append(
            {
                "hsp": hsp[b],
                "wqp": wqp,
                "wkp": wkp,
                "wvp": wvp,
                "wop": wop,
                "cosp": cosp,
                "sinp": sinp,
                "invq": invq,
                "invk": invk,
                "onec": onec,
                "oner": oner,
                "epsb": epsb,
            }
        )
    return in_maps


_PROGRAM = None


def kernel(hidden_states, wq, wk, wv, wo, q_norm_w, k_norm_w):
    global _PROGRAM
    from concourse.bass_utils import run_bass_kernel_spmd

    if _PROGRAM is None:
        _PROGRAM = build_program()
    in_maps = _host_inputs(hidden_states, wq, wk, wv, wo, q_norm_w, k_norm_w)
    res = run_bass_kernel_spmd(_PROGRAM, in_maps, core_ids=list(range(8)))
    out = np.empty((B, S, HID), np.float32)
    for core in range(8):
        b, h = core // 4, core % 4
        out[b, h * 512 : (h + 1) * 512, :] = res.results[core]["out"]
    return out


# revision 17
# speedup vs baseline: 1.4281x; 1.1570x over previous
"""LyraGemma3 sliding-window attention — Trainium2 Bass kernel, 8 NeuronCores.

Sharding: core = b*4 + h  (b in {0,1} batch, h in {0..3} head-group).
Each core owns vanilla head h, lyra head 4+h, kv head h for batch b and
produces output rows [512h, 512h+512) of batch b — disjoint slices, no
collectives.

v2 design (vs the f32r/DRAM-spill baseline):
- All matmul operands bf16 (1 cycle/row, half the DMA bytes, half SBUF).
- q/k/v and outC stay SBUF-resident between phases (no DRAM spill, no
  phase barrier: single program order A -> C -> D keeps PE dense).
- Host prepacks every DRAM tensor into its SBUF layout so all DMAs are
  contiguous (no transposing descriptors).
- Sliding-window/causal masks applied multiplicatively after exp via
  GpSimd affine_select (zero-fill), removing the DVE mask adds and the
  mask tiles entirely.
- PSUM->SBUF copies and rms-norm rsqrt run on the Scalar (ACT) engine;
  rstd/softmax-denominator broadcasts run on GpSimd partition_broadcast,
  keeping DVE for rope only.
- wo streams into SBUF during phase C (address space reused from the
  phase-A weights), so phase D is pure PE work.
"""

import sys

sys.path.insert(0, "/opt/trn_rl_repo")

import numpy as np
import ml_dtypes

import concourse.bass as bass
import concourse.tile as tile
from concourse import mybir
from concourse.tile import ScopedClock

F32 = mybir.dt.float32
BF16 = mybir.dt.bfloat16
AF = mybir.ActivationFunctionType
ALU = mybir.AluOpType
NPBF = ml_dtypes.bfloat16

B, S, HID = 2, 2048, 2560
H, KV, D = 8, 4, 256
WINDOW = 1024
THETA = 10000.0
EPS = 1e-6
SCALING = 256.0 ** (-0.5)  # 1/16

NKC = HID // 128  # 20 contraction chunks for projections
NST = 8           # phase-A s-tiles of 256 tokens
NT = S // 128     # 16 key tiles of 128
NQ = 4            # attention q-tiles of 512


class SplitWaitTC(tile.TileContext):
    """This container's walrus encodes at most ONE semaphore wait per
    instruction; Tile emits multi-wait sync_info. Hoist extra waits onto
    preceding same-engine NOPs."""

    def _drain_and_barrier(self, tick_clock, wait_clock):
        nc = self.nc
        drain_inst = nc.sync.drain()
        wait_clock.add_sem_waits(
            drain_inst.ins, ScopedClock({None: tick_clock.global_clock})
        )
        self._split_multi_waits()
        nc.all_engine_barrier()
        popped = nc._tile_sem_poison_stack.pop()
        assert popped is self._sem_poison
        nc.clear_and_free_semaphores(list(self.sems.allocated().values()))
        nc.all_engine_barrier()

    def _split_multi_waits(self):
        nc = self.nc
        cur_bb = nc.cur_bb
        assert cur_bb is not None
        for f in nc.m.functions:
            for blk in f.blocks:
                insts = blk.instructions
                i = 0
                while i < len(insts):
                    inst = insts[i]
                    si = inst.sync_info
                    if si is not None and si.on_wait and len(si.on_wait) > 1:
                        waits = list(si.on_wait)
                        inst.sync_info = mybir.SyncInfo(
                            on_wait=waits[-1:], on_update=si.on_update
                        )
                        eng = inst.engine
                        for w in waits[:-1]:
                            nop = nc.engines[eng].nop()
                            nop.ins.sync_info = mybir.SyncInfo(
                                on_wait=[w], on_update=[]
                            )
                            cur_bb.bb.instructions.remove(nop.ins)
                            insts.insert(i, nop.ins)
                            i += 1
                    i += 1


def _mask_kind(T, Q):
    """Classify key-tile T against q-tile Q (queries [512Q, 512Q+512)).
    Returns None (fully valid), ('causal', ...) or ('window', ...)."""
    if T >= 4 * Q:
        return "causal"
    if T >= 4 * Q - 4:
        return None
    return "window"


def build_program():
    nc = bass.Bass()

    hsp = nc.declare_dram_parameter("hsp", [128, NST * NKC * 256], BF16, isOutput=False)
    wqp = nc.declare_dram_parameter("wqp", [4, 128, NKC * 128], BF16, isOutput=False)
    wkp = nc.declare_dram_parameter("wkp", [2, 128, NKC * 128], BF16, isOutput=False)
    wvp = nc.declare_dram_parameter("wvp", [128, NKC * 256], BF16, isOutput=False)
    wop = nc.declare_dram_parameter("wop", [128, 16 * HID], BF16, isOutput=False)
    cosp = nc.declare_dram_parameter("cosp", [128, S], BF16, isOutput=False)
    sinp = nc.declare_dram_parameter("sinp", [128, S], BF16, isOutput=False)
    invq_d = nc.declare_dram_parameter("invq", [128, 2], BF16, isOutput=False)
    invk_d = nc.declare_dram_parameter("invk", [128, 2], BF16, isOutput=False)
    onec_d = nc.declare_dram_parameter("onec", [128, 1], BF16, isOutput=False)
    oner_d = nc.declare_dram_parameter("oner", [1, 128], BF16, isOutput=False)
    epsb_d = nc.declare_dram_parameter("epsb", [1, 1], F32, isOutput=False)
    out_d = nc.declare_dram_parameter("out", [512, HID], F32, isOutput=True)

    with SplitWaitTC(nc) as tc:
        with tc.tile_pool(name="outer", bufs=1) as pO:
            onec = pO.tile([128, 1], BF16, name="onec")
            nc.sync.dma_start(onec[:], onec_d[:])
            oner = pO.tile([1, 128], BF16, name="oner")
            nc.sync.dma_start(oner[:], oner_d[:])
            epsb = pO.tile([1, 1], F32, name="epsb")
            nc.sync.dma_start(epsb[:], epsb_d[:])
            invq = pO.tile([128, 2], BF16, name="invq")
            nc.sync.dma_start(invq[:], invq_d[:])
            invk = pO.tile([128, 2], BF16, name="invk")
            nc.sync.dma_start(invk[:], invk_d[:])

            # persistent intermediates (bf16, SBUF-resident across phases)
            qT = pO.tile([128, 4 * S], BF16, name="qT")
            kTr = pO.tile([128, 2 * S], BF16, name="kTr")
            kTn = pO.tile([128, 2 * S], BF16, name="kTn")
            v_all = pO.tile([128, NT * 256], BF16, name="v_all")
            outC = [
                [pO.tile([128, S], BF16, name=f"outC{s}{c}") for c in range(2)]
                for s in range(2)
            ]

            zero_fill = nc.gpsimd.to_reg(0.0)

            # ================= PHASE A: projections + norm + rope ========
            with (
                tc.tile_pool(name="pA", bufs=1) as pA,
                tc.tile_pool(name="pAps", bufs=1, space="PSUM") as psA,
            ):
                wq_sb = [pA.tile([128, NKC * 128], BF16, name=f"wq{hc}") for hc in range(4)]
                nc.sync.dma_start(wq_sb[0][:], wqp[0])
                hst_t = {}

                def hst_dma(st):
                    t = pA.tile([128, NKC * 256], BF16, name="hst", bufs=2)
                    nc.sync.dma_start(t[:], hsp[:, st * NKC * 256 : (st + 1) * NKC * 256])
                    hst_t[st] = t

                hst_dma(0)
                for hc in range(1, 4):
                    nc.sync.dma_start(wq_sb[hc][:], wqp[hc])
                wk_sb = [pA.tile([128, NKC * 128], BF16, name=f"wk{hc}") for hc in range(2)]
                for hc in range(2):
                    nc.sync.dma_start(wk_sb[hc][:], wkp[hc])
                wv_sb = pA.tile([128, NKC * 256], BF16, name="wv_sb")
                nc.sync.dma_start(wv_sb[:], wvp[:])
                hst_dma(1)
                cos_sb = pA.tile([128, S], BF16, name="cos_sb")
                nc.sync.dma_start(cos_sb[:], cosp[:])
                sin_sb = pA.tile([128, S], BF16, name="sin_sb")
                nc.sync.dma_start(sin_sb[:], sinp[:])

                prev_tail = None
                for st in range(NST):
                    s0 = st * 256
                    hst = hst_t[st]
                    if st + 2 < NST:
                        hst_dma(st + 2)
                    # ---- projections (accumulate over 20 HID chunks) ----
                    qz = pA.tile([128, 1024], BF16, name="qz", bufs=2)
                    for hc in range(4):
                        pq = psA.tile([128, 256], F32, name="pacc", bufs=3)
                        for kc in range(NKC):
                            nc.tensor.matmul(
                                pq[:],
                                wq_sb[hc][:, kc * 128 : (kc + 1) * 128],
                                hst[:, kc * 256 : (kc + 1) * 256],
                                start=(kc == 0),
                                stop=(kc == NKC - 1),
                            )
                        nc.scalar.copy(qz[:, hc * 256 : (hc + 1) * 256], pq[:])
                    # square as soon as qz is complete so pn never stalls PE
                    sqq = pA.tile([128, 1024], BF16, name="sqq", bufs=2)
                    nc.scalar.activation(sqq[:], qz[:], AF.Square)
                    kz = pA.tile([128, 512], BF16, name="kz", bufs=2)
                    for hc in range(2):
                        pk = psA.tile([128, 256], F32, name="pacc", bufs=3)
                        for kc in range(NKC):
                            nc.tensor.matmul(
                                pk[:],
                                wk_sb[hc][:, kc * 128 : (kc + 1) * 128],
                                hst[:, kc * 256 : (kc + 1) * 256],
                                start=(kc == 0),
                                stop=(kc == NKC - 1),
                            )
                        nc.scalar.copy(kz[:, hc * 256 : (hc + 1) * 256], pk[:])
                    sqk = pA.tile([128, 512], BF16, name="sqk", bufs=2)
                    nc.scalar.activation(sqk[:], kz[:], AF.Square)
                    for sm in range(2):
                        pv = psA.tile([128, 256], F32, name="pacc", bufs=3)
                        for kc in range(NKC):
                            nc.tensor.matmul(
                                pv[:],
                                hst[:, kc * 256 + sm * 128 : kc * 256 + sm * 128 + 128],
                                wv_sb[:, kc * 256 : (kc + 1) * 256],
                                start=(kc == 0),
                                stop=(kc == NKC - 1),
                            )
                        nc.scalar.copy(
                            v_all[:, (2 * st + sm) * 256 : (2 * st + sm + 1) * 256],
                            pv[:],
                        )

                    # ---- rms-norm sumsq (3 heads: qv, ql, k) ----
                    pns = []
                    for head in range(3):  # 0: q vanilla, 1: q lyra, 2: k
                        pn = psA.tile([1, 256], F32, name="pn", bufs=2)
                        for c in range(2):
                            if head < 2:
                                rhs = sqq[:, (head * 2 + c) * 256 : (head * 2 + c + 1) * 256]
                                lhsT = invq[:, c : c + 1]
                            else:
                                rhs = sqk[:, c * 256 : (c + 1) * 256]
                                lhsT = invk[:, c : c + 1]
                            nc.tensor.matmul(
                                pn[:], lhsT, rhs, start=(c == 0), stop=(c == 1)
                            )
                        srt = pA.tile([1, 256], F32, name="srt", bufs=2)
                        nc.scalar.activation(
                            srt[:], pn[:], AF.Sqrt, bias=epsb[:], scale=1.0 / 256.0
                        )
                        rst = pA.tile([1, 256], BF16, name="rst", bufs=2)
                        with nc.allow_low_precision(reason="rms rstd"):
                            nc.vector.reciprocal(rst[:], srt[:])
                        pns.append(rst)

                    # tail (broadcast + rope) for the PREVIOUS s-tile: its
                    # norm chain has long finished, so the pbc matmuls never
                    # stall PE, and rope (DVE) runs under this tile's
                    # projections.
                    def make_tail(st, s0, qz, kz, rsts):
                        def tail():
                            bcs = []
                            for head in range(3):
                                pbc = psA.tile([128, 256], F32, name="pbc", bufs=2)
                                nc.tensor.matmul(
                                    pbc[:], oner[:], rsts[head][:], start=True, stop=True
                                )
                                bc = pA.tile([128, 256], BF16, name=f"bc{head}", bufs=2)
                                nc.scalar.copy(bc[:], pbc[:])
                                bcs.append(bc)
                            cs = cos_sb[:, s0 : s0 + 256]
                            sn = sin_sb[:, s0 : s0 + 256]

                            def rope2(z0, z1, bc, d0, d1):
                                t0 = pA.tile([128, 256], BF16, name="t0", bufs=2)
                                nc.vector.tensor_mul(t0[:], z0, cs)
                                t1 = pA.tile([128, 256], BF16, name="t1", bufs=2)
                                nc.vector.tensor_mul(t1[:], z1, sn)
                                u0 = pA.tile([128, 256], BF16, name="u0", bufs=2)
                                nc.vector.tensor_sub(u0[:], t0[:], t1[:])
                                nc.vector.tensor_mul(d0, u0[:], bc[:])
                                t2 = pA.tile([128, 256], BF16, name="t2", bufs=2)
                                nc.vector.tensor_mul(t2[:], z1, cs)
                                t3 = pA.tile([128, 256], BF16, name="t3", bufs=2)
                                nc.vector.tensor_mul(t3[:], z0, sn)
                                u1 = pA.tile([128, 256], BF16, name="u1", bufs=2)
                                nc.vector.tensor_add(u1[:], t2[:], t3[:])
                                nc.vector.tensor_mul(d1, u1[:], bc[:])

                            for head in range(2):
                                rope2(
                                    qz[:, (head * 2) * 256 : (head * 2) * 256 + 256],
                                    qz[:, (head * 2 + 1) * 256 : (head * 2 + 1) * 256 + 256],
                                    bcs[head],
                                    qT[:, (head * 2) * S + s0 : (head * 2) * S + s0 + 256],
                                    qT[:, (head * 2 + 1) * S + s0 : (head * 2 + 1) * S + s0 + 256],
                                )
                            rope2(
                                kz[:, 0:256], kz[:, 256:512], bcs[2],
                                kTr[:, s0 : s0 + 256], kTr[:, S + s0 : S + s0 + 256],
                            )
                            nc.vector.tensor_mul(
                                kTn[:, s0 : s0 + 256], kz[:, 0:256], bcs[2][:]
                            )
                            nc.vector.tensor_mul(
                                kTn[:, S + s0 : S + s0 + 256], kz[:, 256:512], bcs[2][:]
                            )

                        return tail

                    if prev_tail is not None:
                        prev_tail()
                    prev_tail = make_tail(st, s0, qz, kz, pns)
                prev_tail()

            # ================= PHASES C+D ================================
            # wo streams into the SBUF space the phase-A pool released.
            with tc.tile_pool(name="pWo", bufs=1) as pWo:
                wo_sb = pWo.tile([128, 16 * HID], BF16, name="wo_sb")
                for i in range(4):
                    nc.sync.dma_start(
                        wo_sb[:, i * 4 * HID : (i + 1) * 4 * HID],
                        wop[:, i * 4 * HID : (i + 1) * 4 * HID],
                    )

                # ================= PHASE C: attention ====================
                with (
                    tc.tile_pool(name="pC", bufs=1) as pC,
                    tc.tile_pool(name="pCps", bufs=1, space="PSUM") as psC,
                ):
                    prev_norm = None
                    for stream in range(2):  # 0 = vanilla (roped k), 1 = lyra
                        kT = kTr if stream == 0 else kTn
                        q0 = (2 * stream) * S
                        q1 = (2 * stream + 1) * S
                        for Q in range(NQ):
                            T_lo = max(0, 4 * Q - 8)
                            T_hi = 4 * Q + 3
                            po0 = psC.tile([128, 512], F32, name="po0", bufs=2)
                            po1 = psC.tile([128, 512], F32, name="po1", bufs=2)
                            psm = psC.tile([1, 512], F32, name="psm", bufs=1)
                            probs_t = {}

                            def emit_scores(T):
                                pss = psC.tile([128, 512], F32, name="pss", bufs=2)
                                nc.tensor.matmul(
                                    pss[:],
                                    kT[:, T * 128 : (T + 1) * 128],
                                    qT[:, q0 + Q * 512 : q0 + (Q + 1) * 512],
                                    start=True,
                                    stop=False,
                                )
                                nc.tensor.matmul(
                                    pss[:],
                                    kT[:, S + T * 128 : S + (T + 1) * 128],
                                    qT[:, q1 + Q * 512 : q1 + (Q + 1) * 512],
                                    start=False,
                                    stop=True,
                                )
                                kind = _mask_kind(T, Q)
                                probs = pC.tile([128, 512], BF16, name="probs", bufs=4)
                                if kind is None:
                                    nc.scalar.activation(
                                        probs[:], pss[:], AF.Exp, scale=SCALING
                                    )
                                elif kind == "causal":
                                    praw = pC.tile([128, 512], BF16, name="praw", bufs=2)
                                    nc.scalar.activation(
                                        praw[:], pss[:], AF.Exp, scale=SCALING
                                    )
                                    # keep where 512Q + f >= 128T + p
                                    nc.gpsimd.affine_select(
                                        probs[:], praw[:],
                                        pattern=[[1, 512]],
                                        compare_op=ALU.is_ge,
                                        fill=zero_fill,
                                        base=512 * Q - 128 * T,
                                        channel_multiplier=-1,
                                    )
                                else:
                                    praw = pC.tile([128, 512], BF16, name="praw", bufs=2)
                                    nc.scalar.activation(
                                        praw[:], pss[:], AF.Exp, scale=SCALING
                                    )
                                    # keep where 128T + p > 512Q + f - 1024
                                    nc.gpsimd.affine_select(
                                        probs[:], praw[:],
                                        pattern=[[-1, 512]],
                                        compare_op=ALU.is_ge,
                                        fill=zero_fill,
                                        base=128 * T - 512 * Q + 1023,
                                        channel_multiplier=1,
                                    )
                                probs_t[T] = probs

                            def emit_av(T):
                                probs = probs_t.pop(T)
                                first = T == T_lo
                                last = T == T_hi
                                nc.tensor.matmul(
                                    psm[:], onec[:], probs[:], start=first, stop=last
                                )
                                nc.tensor.matmul(
                                    po0[:],
                                    v_all[:, T * 256 : T * 256 + 128],
                                    probs[:],
                                    start=first,
                                    stop=last,
                                )
                                nc.tensor.matmul(
                                    po1[:],
                                    v_all[:, T * 256 + 128 : T * 256 + 256],
                                    probs[:],
                                    start=first,
                                    stop=last,
                                )

                            # software pipeline: AV for tile T trails the
                            # scores for tile T+2 so PE never waits on the
                            # ACT exp / GpSimd mask chain; the previous
                            # q-tile's normalize chain is emitted after the
                            # first scores block for the same reason.
                            for idx, T in enumerate(range(T_lo, T_hi + 1)):
                                emit_scores(T)
                                if idx == 0 and prev_norm is not None:
                                    prev_norm()
                                if idx >= 2:
                                    emit_av(T - 2)
                            emit_av(T_hi - 1)
                            emit_av(T_hi)
                            # free the single psm bank ASAP (ACT copy); the
                            # rest of the normalize chain is deferred into
                            # the next q-tile's score stream.
                            psmb = pC.tile([1, 512], BF16, name="psmb", bufs=2)
                            nc.scalar.copy(psmb[:], psm[:])

                            def make_norm(stream, Q, po0, po1, psmb):
                                def norm():
                                    pbcC = psC.tile(
                                        [128, 512], F32, name="pbcC", bufs=1
                                    )
                                    nc.tensor.matmul(
                                        pbcC[:], oner[:], psmb[:], start=True, stop=True
                                    )
                                    bcsC = pC.tile([128, 512], F32, name="bcsC", bufs=2)
                                    nc.vector.reciprocal(bcsC[:], pbcC[:])
                                    for dc in range(2):
                                        po = po0 if dc == 0 else po1
                                        nc.vector.tensor_mul(
                                            outC[stream][dc][:, Q * 512 : (Q + 1) * 512],
                                            po[:],
                                            bcsC[:],
                                        )

                                return norm

                            prev_norm = make_norm(stream, Q, po0, po1, psmb)
                    prev_norm()

                # ================= PHASE D: output projection ============
                with (
                    tc.tile_pool(name="pD", bufs=1) as pD,
                    tc.tile_pool(name="pDps", bufs=1, space="PSUM") as psD,
                ):
                    # outC is stored contiguously in query order; the lhsT
                    # for contraction chunk (j, dc) is the stride-8 view
                    # q = 8m + j over rows m0..m0+128.
                    for co in range(0, HID, 512):
                        pos = [
                            psD.tile([128, 512], F32, name=f"pD{m}", bufs=2)
                            for m in range(4)
                        ]
                        for kc in range(16):
                            j, dc = kc // 2, kc % 2
                            for m in range(4):
                                stream, m0 = m // 2, (m % 2) * 128
                                lhsT = (
                                    outC[stream][dc][:]
                                    .rearrange("p (m j) -> p m j", j=8)
                                    [:, m0 : m0 + 128, j : j + 1]
                                )
                                nc.tensor.matmul(
                                    pos[m][:],
                                    lhsT,
                                    wo_sb[:, kc * HID + co : kc * HID + co + 512],
                                    start=(kc == 0),
                                    stop=(kc == 15),
                                )
                        for m in range(4):
                            ost = pD.tile([128, 512], F32, name="ost", bufs=3)
                            nc.scalar.copy(ost[:], pos[m][:])
                            nc.sync.dma_start(
                                out_d[m * 128 : (m + 1) * 128, co : co + 512],
                                ost[:],
                            )
    return nc


def _host_inputs(hidden_states, wq, wk, wv, wo, q_norm_w, k_norm_w):
    """Build the 8 per-core input maps (all host-side numpy prep).
    Every tensor is prepacked into its exact SBUF layout so device DMAs
    are plain contiguous copies."""
    hs = np.asarray(hidden_states, dtype=np.float32)
    wq = np.asarray(wq, dtype=np.float32)
    wk = np.asarray(wk, dtype=np.float32)
    wv = np.asarray(wv, dtype=np.float32)
    wo = np.asarray(wo, dtype=np.float32)
    qnw = np.asarray(q_norm_w, dtype=np.float32)
    knw = np.asarray(k_norm_w, dtype=np.float32)

    def pack_w(w):  # [HID, width] -> [128, NKC*width] chunk-major free axis
        width = w.shape[1]
        return np.ascontiguousarray(
            w.reshape(NKC, 128, width).transpose(1, 0, 2).reshape(128, NKC * width)
        ).astype(NPBF)

    # hsT packed per s-tile: [128, (st, kc, 256)]
    hsp = []
    for b in range(B):
        h = hs[b].T.reshape(NKC, 128, NST, 256).transpose(1, 2, 0, 3)
        hsp.append(np.ascontiguousarray(h.reshape(128, NST * NKC * 256)).astype(NPBF))

    inv_freq = 1.0 / (THETA ** (np.arange(0, D, 2, dtype=np.float32) / D))
    ang = np.outer(inv_freq, np.arange(S, dtype=np.float32))  # (128, S)
    cosp = np.ascontiguousarray(np.cos(ang)).astype(NPBF)
    sinp = np.ascontiguousarray(np.sin(ang)).astype(NPBF)

    invq = np.ascontiguousarray(((1.0 + qnw) ** -2).reshape(2, 128).T).astype(NPBF)
    invk = np.ascontiguousarray(((1.0 + knw) ** -2).reshape(2, 128).T).astype(NPBF)
    onec = np.ones((128, 1), NPBF)
    oner = np.ones((1, 128), NPBF)
    epsb = np.full((1, 1), EPS, np.float32)

    # wo packed: [128, (kc, HID)]
    wop = np.ascontiguousarray(
        wo.reshape(16, 128, HID).transpose(1, 0, 2).reshape(128, 16 * HID)
    ).astype(NPBF)

    qs = 1.0 + qnw
    ks = 1.0 + knw
    in_maps = []
    for core in range(8):
        b, h = core // 4, core % 4
        wq2 = np.concatenate(
            [
                wq[:, h * D : (h + 1) * D] * qs[None, :],
                wq[:, (4 + h) * D : (5 + h) * D] * qs[None, :],
            ],
            axis=1,
        )  # [HID, 512]
        wqp = np.stack([pack_w(wq2[:, hc * 128 : (hc + 1) * 128]) for hc in range(4)])
        wk1 = wk[:, h * D : (h + 1) * D] * ks[None, :]
        wkp = np.stack([pack_w(wk1[:, hc * 128 : (hc + 1) * 128]) for hc in range(2)])
        wvp = pack_w(wv[:, h * D : (h + 1) * D])
        in_maps.append(
            {
                "hsp": hsp[b],
                "wqp": wqp,
                "wkp": wkp,
                "wvp": wvp,
                "wop": wop,
                "cosp": cosp,
                "sinp": sinp,
                "invq": invq,
                "invk": invk,
                "onec": onec,
                "oner": oner,
                "epsb": epsb,
            }
        )
    return in_maps


_PROGRAM = None


def kernel(hidden_states, wq, wk, wv, wo, q_norm_w, k_norm_w):
    global _PROGRAM
    from concourse.bass_utils import run_bass_kernel_spmd

    if _PROGRAM is None:
        _PROGRAM = build_program()
    in_maps = _host_inputs(hidden_states, wq, wk, wv, wo, q_norm_w, k_norm_w)
    res = run_bass_kernel_spmd(_PROGRAM, in_maps, core_ids=list(range(8)))
    out = np.empty((B, S, HID), np.float32)
    for core in range(8):
        b, h = core // 4, core % 4
        out[b, h * 512 : (h + 1) * 512, :] = res.results[core]["out"]
    return out


# revision 20
# speedup vs baseline: 1.5729x; 1.1014x over previous
"""LyraGemma3 sliding-window attention — Trainium2 Bass kernel, 8 NeuronCores.

Sharding: core = b*4 + h  (b in {0,1} batch, h in {0..3} head-group).
Each core owns vanilla head h, lyra head 4+h, kv head h for batch b and
produces output rows [512h, 512h+512) of batch b — disjoint slices, no
collectives.

v4 design:
- All matmul operands bf16; q/k/v and outC SBUF-resident (no DRAM spill).
- Persistent intermediates split per 512-token group so phase C's first
  q-tiles depend only on early phase-A groups (no whole-tile false deps).
- Masks applied multiplicatively after exp via GpSimd affine_select, and
  masked tiles compute only their valid query subrange (causal tiles
  shrink, window-edge tiles grow), with the T loop ordered so a
  full-range tile carries the PSUM-zeroing start flag.
- 1/x and 1/sqrt(x) computed as exp(-ln(x)) / exp(-0.5 ln(x)) on the
  Scalar engine: every activation (Square/Copy/Ln/Exp) lives in one ACT
  table, and the slow DVE reciprocal disappears from all critical paths.
- Phase A/C tails (rstd broadcast + rope, softmax normalize) are
  deferred into the next tile's instruction stream so PE never waits.
- wo streams into SBUF during phase C (address space reused from the
  phase-A weights); phase D runs m-block-major with deferred PSUM->SBUF
  copies so only the last output block's copy+DMA is exposed.
"""

import sys

sys.path.insert(0, "/opt/trn_rl_repo")

import numpy as np
import ml_dtypes

import concourse.bass as bass
import concourse.tile as tile
from concourse import mybir
from concourse.tile import ScopedClock

F32 = mybir.dt.float32
BF16 = mybir.dt.bfloat16
AF = mybir.ActivationFunctionType
ALU = mybir.AluOpType
NPBF = ml_dtypes.bfloat16

B, S, HID = 2, 2048, 2560
H, KV, D = 8, 4, 256
WINDOW = 1024
THETA = 10000.0
EPS = 1e-6
SCALING = 256.0 ** (-0.5)  # 1/16

NKC = HID // 128  # 20 contraction chunks for projections
NST = 8           # phase-A s-tiles of 256 tokens
NT = S // 128     # 16 key tiles of 128
NQ = 4            # attention q-tiles of 512


class SplitWaitTC(tile.TileContext):
    """This container's walrus encodes at most ONE semaphore wait per
    instruction; Tile emits multi-wait sync_info. Hoist extra waits onto
    preceding same-engine NOPs."""

    def _drain_and_barrier(self, tick_clock, wait_clock):
        nc = self.nc
        drain_inst = nc.sync.drain()
        wait_clock.add_sem_waits(
            drain_inst.ins, ScopedClock({None: tick_clock.global_clock})
        )
        self._split_multi_waits()
        nc.all_engine_barrier()
        popped = nc._tile_sem_poison_stack.pop()
        assert popped is self._sem_poison
        nc.clear_and_free_semaphores(list(self.sems.allocated().values()))
        nc.all_engine_barrier()

    def _split_multi_waits(self):
        nc = self.nc
        cur_bb = nc.cur_bb
        assert cur_bb is not None
        for f in nc.m.functions:
            for blk in f.blocks:
                insts = blk.instructions
                i = 0
                while i < len(insts):
                    inst = insts[i]
                    si = inst.sync_info
                    if si is not None and si.on_wait and len(si.on_wait) > 1:
                        waits = list(si.on_wait)
                        inst.sync_info = mybir.SyncInfo(
                            on_wait=waits[-1:], on_update=si.on_update
                        )
                        eng = inst.engine
                        for w in waits[:-1]:
                            nop = nc.engines[eng].nop()
                            nop.ins.sync_info = mybir.SyncInfo(
                                on_wait=[w], on_update=[]
                            )
                            cur_bb.bb.instructions.remove(nop.ins)
                            insts.insert(i, nop.ins)
                            i += 1
                    i += 1


def _c_tiles(Q):
    """Key-tile schedule for q-tile Q (queries [512Q, 512Q+512)).
    Returns [(T, off, length, select)] where [off, off+length) is the
    valid query subrange and select is None or (pattern, base, chan_mult)
    for the post-exp GpSimd affine_select. Ordered so the first entry is
    full-range (its matmul carries start=True and zeroes the whole PSUM
    region)."""
    out = []
    for T in range(max(0, 4 * Q - 4), 4 * Q):  # fully-valid tiles
        out.append((T, 0, 512, None))
    for j in range(4):  # causal diagonal: queries f >= 128 j are live
        ln = 512 - 128 * j
        # keep where f' - p >= 0 (f' is the index within the subrange)
        out.append((4 * Q + j, 128 * j, ln, ([[1, ln]], 0, -1)))
    if Q >= 2:
        for jp in range(4):  # window edge: queries f <= 128 jp + 126 live
            ln = 128 * jp + 128
            # keep where p - f + (128 jp - 1) >= 0
            out.append((4 * Q - 8 + jp, 0, ln, ([[-1, ln]], 128 * jp - 1, 1)))
    return out


def build_program():
    nc = bass.Bass()

    hsp = nc.declare_dram_parameter("hsp", [128, NST * NKC * 256], BF16, isOutput=False)
    wqp = nc.declare_dram_parameter("wqp", [4, 128, NKC * 128], BF16, isOutput=False)
    wkp = nc.declare_dram_parameter("wkp", [2, 128, NKC * 128], BF16, isOutput=False)
    wvp = nc.declare_dram_parameter("wvp", [128, NKC * 256], BF16, isOutput=False)
    wop = nc.declare_dram_parameter("wop", [128, 16 * HID], BF16, isOutput=False)
    cosp = nc.declare_dram_parameter("cosp", [128, S], BF16, isOutput=False)
    sinp = nc.declare_dram_parameter("sinp", [128, S], BF16, isOutput=False)
    invq_d = nc.declare_dram_parameter("invq", [128, 2], BF16, isOutput=False)
    invk_d = nc.declare_dram_parameter("invk", [128, 2], BF16, isOutput=False)
    onec_d = nc.declare_dram_parameter("onec", [128, 1], BF16, isOutput=False)
    oner_d = nc.declare_dram_parameter("oner", [1, 128], BF16, isOutput=False)
    epsb_d = nc.declare_dram_parameter("epsb", [1, 1], F32, isOutput=False)
    out_d = nc.declare_dram_parameter("out", [512, HID], F32, isOutput=True)

    with SplitWaitTC(nc) as tc:
        with tc.tile_pool(name="outer", bufs=1) as pO:
            onec = pO.tile([128, 1], BF16, name="onec")
            nc.sync.dma_start(onec[:], onec_d[:])
            oner = pO.tile([1, 128], BF16, name="oner")
            nc.sync.dma_start(oner[:], oner_d[:])
            epsb = pO.tile([1, 1], F32, name="epsb")
            nc.sync.dma_start(epsb[:], epsb_d[:])
            invq = pO.tile([128, 2], BF16, name="invq")
            nc.sync.dma_start(invq[:], invq_d[:])
            invk = pO.tile([128, 2], BF16, name="invk")
            nc.sync.dma_start(invk[:], invk_d[:])

            # persistent intermediates, split per 512-token group g:
            # qTs[g]: [128, c*512 + t], c in {van d0, van d1, lyra d0, lyra d1}
            # kTrs/kTns[g]: [128, c*512 + t], c = d half
            # v_s[g]: [128, tloc*256 + d], tloc = key-tile within group
            qTs = [pO.tile([128, 4 * 512], BF16, name=f"qT{g}") for g in range(4)]
            kTrs = [pO.tile([128, 2 * 512], BF16, name=f"kTr{g}") for g in range(4)]
            kTns = [pO.tile([128, 2 * 512], BF16, name=f"kTn{g}") for g in range(4)]
            v_s = [pO.tile([128, 4 * 256], BF16, name=f"v{g}") for g in range(4)]
            outC = [
                [pO.tile([128, S], BF16, name=f"outC{s}{c}") for c in range(2)]
                for s in range(2)
            ]

            zero_fill = nc.gpsimd.to_reg(0.0)

            # ================= PHASE A: projections + norm + rope ========
            with (
                tc.tile_pool(name="pA", bufs=1) as pA,
                tc.tile_pool(name="pAps", bufs=1, space="PSUM") as psA,
            ):
                # s-tile 0 arrives as four quarter tiles so the first
                # matmuls start after ~1MB of DMA instead of ~4MB.
                hst0q = [
                    pA.tile([128, 5 * 256], BF16, name=f"hst0q{i}") for i in range(4)
                ]
                wq_sb = [pA.tile([128, NKC * 128], BF16, name=f"wq{hc}") for hc in range(4)]
                nc.sync.dma_start(hst0q[0][:], hsp[:, 0 : 5 * 256])
                nc.sync.dma_start(wq_sb[0][:], wqp[0])
                for i in range(1, 4):
                    nc.sync.dma_start(
                        hst0q[i][:], hsp[:, i * 5 * 256 : (i + 1) * 5 * 256]
                    )
                for hc in range(1, 4):
                    nc.sync.dma_start(wq_sb[hc][:], wqp[hc])
                wk_sb = [pA.tile([128, NKC * 128], BF16, name=f"wk{hc}") for hc in range(2)]
                for hc in range(2):
                    nc.sync.dma_start(wk_sb[hc][:], wkp[hc])
                wv_sb = pA.tile([128, NKC * 256], BF16, name="wv_sb")
                nc.sync.dma_start(wv_sb[:], wvp[:])
                hst_t = {}

                def hst_dma(st):
                    t = pA.tile([128, NKC * 256], BF16, name="hst", bufs=3)
                    nc.sync.dma_start(t[:], hsp[:, st * NKC * 256 : (st + 1) * NKC * 256])
                    hst_t[st] = t

                hst_dma(1)
                cos_sb = pA.tile([128, S], BF16, name="cos_sb")
                nc.sync.dma_start(cos_sb[:], cosp[:])
                sin_sb = pA.tile([128, S], BF16, name="sin_sb")
                nc.sync.dma_start(sin_sb[:], sinp[:])

                def hs_ap(st, kc, lo, width):
                    if st == 0:
                        t = hst0q[kc // 5]
                        base = (kc % 5) * 256
                        return t[:, base + lo : base + lo + width]
                    return hst_t[st][:, kc * 256 + lo : kc * 256 + lo + width]

                prev_tail = None
                for st in range(NST):
                    s0 = st * 256
                    g, off = st // 2, (st % 2) * 256
                    if st + 2 < NST:
                        hst_dma(st + 2)
                    # ---- projections (accumulate over 20 HID chunks) ----
                    qz = pA.tile([128, 1024], BF16, name="qz", bufs=2)
                    for hc in range(4):
                        pq = psA.tile([128, 256], F32, name="pacc", bufs=3)
                        for kc in range(NKC):
                            nc.tensor.matmul(
                                pq[:],
                                wq_sb[hc][:, kc * 128 : (kc + 1) * 128],
                                hs_ap(st, kc, 0, 256),
                                start=(kc == 0),
                                stop=(kc == NKC - 1),
                            )
                        nc.scalar.copy(qz[:, hc * 256 : (hc + 1) * 256], pq[:])
                    sqq = pA.tile([128, 1024], BF16, name="sqq", bufs=2)
                    nc.scalar.activation(sqq[:], qz[:], AF.Square)
                    kz = pA.tile([128, 512], BF16, name="kz", bufs=2)
                    for hc in range(2):
                        pk = psA.tile([128, 256], F32, name="pacc", bufs=3)
                        for kc in range(NKC):
                            nc.tensor.matmul(
                                pk[:],
                                wk_sb[hc][:, kc * 128 : (kc + 1) * 128],
                                hs_ap(st, kc, 0, 256),
                                start=(kc == 0),
                                stop=(kc == NKC - 1),
                            )
                        nc.scalar.copy(kz[:, hc * 256 : (hc + 1) * 256], pk[:])
                    sqk = pA.tile([128, 512], BF16, name="sqk", bufs=2)
                    nc.scalar.activation(sqk[:], kz[:], AF.Square)
                    for sm in range(2):
                        pv = psA.tile([128, 256], F32, name="pacc", bufs=3)
                        for kc in range(NKC):
                            nc.tensor.matmul(
                                pv[:],
                                hs_ap(st, kc, sm * 128, 128),
                                wv_sb[:, kc * 256 : (kc + 1) * 256],
                                start=(kc == 0),
                                stop=(kc == NKC - 1),
                            )
                        tl = (2 * st + sm) % 4
                        nc.scalar.copy(
                            v_s[(2 * st + sm) // 4][:, tl * 256 : tl * 256 + 256],
                            pv[:],
                        )

                    # ---- rms rstd = exp(-0.5 ln(msq)), all on ACT ----
                    rsts = []
                    for head in range(3):  # 0: q vanilla, 1: q lyra, 2: k
                        pn = psA.tile([1, 256], F32, name="pn", bufs=2)
                        for c in range(2):
                            if head < 2:
                                rhs = sqq[:, (head * 2 + c) * 256 : (head * 2 + c + 1) * 256]
                                lhsT = invq[:, c : c + 1]
                            else:
                                rhs = sqk[:, c * 256 : (c + 1) * 256]
                                lhsT = invk[:, c : c + 1]
                            nc.tensor.matmul(
                                pn[:], lhsT, rhs, start=(c == 0), stop=(c == 1)
                            )
                        lnm = pA.tile([1, 256], F32, name="lnm", bufs=2)
                        nc.scalar.activation(
                            lnm[:], pn[:], AF.Ln, bias=epsb[:], scale=1.0 / 256.0
                        )
                        rst = pA.tile([1, 256], BF16, name="rst", bufs=2)
                        nc.scalar.activation(rst[:], lnm[:], AF.Exp, scale=-0.5)
                        rsts.append(rst)

                    # tail (rstd broadcast + rope) for the PREVIOUS s-tile:
                    # its norm chain has finished, so the pbc matmuls never
                    # stall PE, and rope (DVE) runs under this tile's
                    # projections.
                    def make_tail(s0, g, off, qz, kz, rsts):
                        def tail():
                            bcs = []
                            for head in range(3):
                                pbc = psA.tile([128, 256], F32, name="pbc", bufs=2)
                                nc.tensor.matmul(
                                    pbc[:], oner[:], rsts[head][:], start=True, stop=True
                                )
                                bc = pA.tile([128, 256], BF16, name=f"bc{head}", bufs=2)
                                nc.scalar.copy(bc[:], pbc[:])
                                bcs.append(bc)
                            cs = cos_sb[:, s0 : s0 + 256]
                            sn = sin_sb[:, s0 : s0 + 256]

                            def rope2(z0, z1, bc, d0, d1):
                                t0 = pA.tile([128, 256], BF16, name="t0", bufs=2)
                                nc.vector.tensor_mul(t0[:], z0, cs)
                                t1 = pA.tile([128, 256], BF16, name="t1", bufs=2)
                                nc.vector.tensor_mul(t1[:], z1, sn)
                                u0 = pA.tile([128, 256], BF16, name="u0", bufs=2)
                                nc.vector.tensor_sub(u0[:], t0[:], t1[:])
                                nc.vector.tensor_mul(d0, u0[:], bc[:])
                                t2 = pA.tile([128, 256], BF16, name="t2", bufs=2)
                                nc.vector.tensor_mul(t2[:], z1, cs)
                                t3 = pA.tile([128, 256], BF16, name="t3", bufs=2)
                                nc.vector.tensor_mul(t3[:], z0, sn)
                                u1 = pA.tile([128, 256], BF16, name="u1", bufs=2)
                                nc.vector.tensor_add(u1[:], t2[:], t3[:])
                                nc.vector.tensor_mul(d1, u1[:], bc[:])

                            for head in range(2):
                                rope2(
                                    qz[:, (head * 2) * 256 : (head * 2) * 256 + 256],
                                    qz[:, (head * 2 + 1) * 256 : (head * 2 + 1) * 256 + 256],
                                    bcs[head],
                                    qTs[g][:, (head * 2) * 512 + off : (head * 2) * 512 + off + 256],
                                    qTs[g][:, (head * 2 + 1) * 512 + off : (head * 2 + 1) * 512 + off + 256],
                                )
                            rope2(
                                kz[:, 0:256], kz[:, 256:512], bcs[2],
                                kTrs[g][:, off : off + 256],
                                kTrs[g][:, 512 + off : 512 + off + 256],
                            )
                            nc.vector.tensor_mul(
                                kTns[g][:, off : off + 256], kz[:, 0:256], bcs[2][:]
                            )
                            nc.vector.tensor_mul(
                                kTns[g][:, 512 + off : 512 + off + 256],
                                kz[:, 256:512],
                                bcs[2][:],
                            )

                        return tail

                    if prev_tail is not None:
                        prev_tail()
                    prev_tail = make_tail(s0, g, off, qz, kz, rsts)
                prev_tail()

            # ================= PHASES C+D ================================
            # wo streams into the SBUF space the phase-A pool released.
            with tc.tile_pool(name="pWo", bufs=1) as pWo:
                wo_sb = pWo.tile([128, 16 * HID], BF16, name="wo_sb")
                for i in range(4):
                    nc.sync.dma_start(
                        wo_sb[:, i * 4 * HID : (i + 1) * 4 * HID],
                        wop[:, i * 4 * HID : (i + 1) * 4 * HID],
                    )

                # ================= PHASE C: attention ====================
                with (
                    tc.tile_pool(name="pC", bufs=1) as pC,
                    tc.tile_pool(name="pCps", bufs=1, space="PSUM") as psC,
                ):
                    prev_norm = None
                    for stream in range(2):  # 0 = vanilla (roped k), 1 = lyra
                        kTg = kTrs if stream == 0 else kTns
                        for Q in range(NQ):
                            tiles = _c_tiles(Q)
                            po0 = psC.tile([128, 512], F32, name="po0", bufs=2)
                            po1 = psC.tile([128, 512], F32, name="po1", bufs=2)
                            psm = psC.tile([1, 512], F32, name="psm", bufs=1)
                            probs_t = {}

                            def emit_scores(i):
                                T, off, ln, sel = tiles[i]
                                kT = kTg[T // 4]
                                tl = (T % 4) * 128
                                pss = psC.tile([128, 512], F32, name="pss", bufs=2)
                                for c in range(2):
                                    qb = (2 * stream + c) * 512 + off
                                    nc.tensor.matmul(
                                        pss[:, off : off + ln],
                                        kT[:, c * 512 + tl : c * 512 + tl + 128],
                                        qTs[Q][:, qb : qb + ln],
                                        start=(c == 0),
                                        stop=(c == 1),
                                    )
                                probs = pC.tile([128, 512], BF16, name="probs", bufs=4)
                                if sel is None:
                                    nc.scalar.activation(
                                        probs[:, off : off + ln],
                                        pss[:, off : off + ln],
                                        AF.Exp,
                                        scale=SCALING,
                                    )
                                else:
                                    pattern, base, cm = sel
                                    praw = pC.tile([128, 512], BF16, name="praw", bufs=2)
                                    nc.scalar.activation(
                                        praw[:, off : off + ln],
                                        pss[:, off : off + ln],
                                        AF.Exp,
                                        scale=SCALING,
                                    )
                                    nc.gpsimd.affine_select(
                                        probs[:, off : off + ln],
                                        praw[:, off : off + ln],
                                        pattern=pattern,
                                        compare_op=ALU.is_ge,
                                        fill=zero_fill,
                                        base=base,
                                        channel_multiplier=cm,
                                    )
                                probs_t[i] = probs

                            def emit_av(i):
                                T, off, ln, _ = tiles[i]
                                probs = probs_t.pop(i)
                                vt = v_s[T // 4]
                                tl = (T % 4) * 256
                                first = i == 0
                                last = i == len(tiles) - 1
                                nc.tensor.matmul(
                                    psm[:, off : off + ln],
                                    onec[:],
                                    probs[:, off : off + ln],
                                    start=first,
                                    stop=last,
                                    skip_group_check=True,
                                )
                                nc.tensor.matmul(
                                    po0[:, off : off + ln],
                                    vt[:, tl : tl + 128],
                                    probs[:, off : off + ln],
                                    start=first,
                                    stop=last,
                                    skip_group_check=True,
                                )
                                nc.tensor.matmul(
                                    po1[:, off : off + ln],
                                    vt[:, tl + 128 : tl + 256],
                                    probs[:, off : off + ln],
                                    start=first,
                                    stop=last,
                                    skip_group_check=True,
                                )

                            # software pipeline: AV for tile i trails the
                            # scores for tile i+2 so PE never waits on the
                            # ACT exp / GpSimd mask chain; the previous
                            # q-tile's normalize chain is emitted after the
                            # first scores block for the same reason.
                            for i in range(len(tiles)):
                                emit_scores(i)
                                if i == 0 and prev_norm is not None:
                                    prev_norm()
                                if i >= 2:
                                    emit_av(i - 2)
                            emit_av(len(tiles) - 2)
                            emit_av(len(tiles) - 1)
                            # free the single psm bank ASAP (ACT copy); the
                            # rest of the normalize chain is deferred into
                            # the next q-tile's score stream.
                            psmb = pC.tile([1, 512], BF16, name="psmb", bufs=2)
                            nc.scalar.copy(psmb[:], psm[:])

                            def make_norm(stream, Q, po0, po1, psmb):
                                def norm():
                                    pbcC = psC.tile(
                                        [128, 512], F32, name="pbcC", bufs=1
                                    )
                                    nc.tensor.matmul(
                                        pbcC[:], oner[:], psmb[:], start=True, stop=True
                                    )
                                    # 1/x as exp(-ln(x)), both on ACT
                                    lnC = pC.tile([128, 512], F32, name="lnC", bufs=2)
                                    nc.scalar.activation(lnC[:], pbcC[:], AF.Ln)
                                    bcsC = pC.tile([128, 512], F32, name="bcsC", bufs=2)
                                    nc.scalar.activation(
                                        bcsC[:], lnC[:], AF.Exp, scale=-1.0
                                    )
                                    for dc in range(2):
                                        po = po0 if dc == 0 else po1
                                        nc.vector.tensor_mul(
                                            outC[stream][dc][:, Q * 512 : (Q + 1) * 512],
                                            po[:],
                                            bcsC[:],
                                        )

                                return norm

                            prev_norm = make_norm(stream, Q, po0, po1, psmb)
                    prev_norm()

                # ================= PHASE D: output projection ============
                with (
                    tc.tile_pool(name="pD", bufs=1) as pD,
                    tc.tile_pool(name="pDps", bufs=1, space="PSUM") as psD,
                ):
                    # outC is stored contiguously in query order; the lhsT
                    # for contraction chunk (j, dc) is the stride-8 view
                    # q = 8m + j over rows m0..m0+128. m-block-major with
                    # copies deferred one block so only the last block's
                    # copy+DMA is exposed.
                    prev_copy = None
                    for co in range(0, HID, 512):
                        for m in range(4):
                            stream, m0 = m // 2, (m % 2) * 128
                            pos = psD.tile([128, 512], F32, name=f"pD{m}", bufs=2)
                            for kc in range(16):
                                j, dc = kc // 2, kc % 2
                                lhsT = (
                                    outC[stream][dc][:]
                                    .rearrange("p (m j) -> p m j", j=8)
                                    [:, m0 : m0 + 128, j : j + 1]
                                )
                                nc.tensor.matmul(
                                    pos[:],
                                    lhsT,
                                    wo_sb[:, kc * HID + co : kc * HID + co + 512],
                                    start=(kc == 0),
                                    stop=(kc == 15),
                                )

                            def make_copy(co, m, pos):
                                def docopy():
                                    ost = pD.tile([128, 512], F32, name="ost", bufs=6)
                                    nc.scalar.copy(ost[:], pos[:])
                                    nc.sync.dma_start(
                                        out_d[m * 128 : (m + 1) * 128, co : co + 512],
                                        ost[:],
                                    )

                                return docopy

                            if prev_copy is not None:
                                prev_copy()
                            prev_copy = make_copy(co, m, pos)
                    prev_copy()
    return nc


def _host_inputs(hidden_states, wq, wk, wv, wo, q_norm_w, k_norm_w):
    """Build the 8 per-core input maps (all host-side numpy prep).
    Every tensor is prepacked into its exact SBUF layout so device DMAs
    are plain contiguous copies."""
    hs = np.asarray(hidden_states, dtype=np.float32)
    wq = np.asarray(wq, dtype=np.float32)
    wk = np.asarray(wk, dtype=np.float32)
    wv = np.asarray(wv, dtype=np.float32)
    wo = np.asarray(wo, dtype=np.float32)
    qnw = np.asarray(q_norm_w, dtype=np.float32)
    knw = np.asarray(k_norm_w, dtype=np.float32)

    def pack_w(w):  # [HID, width] -> [128, NKC*width] chunk-major free axis
        width = w.shape[1]
        return np.ascontiguousarray(
            w.reshape(NKC, 128, width).transpose(1, 0, 2).reshape(128, NKC * width)
        ).astype(NPBF)

    # hsT packed per s-tile: [128, (st, kc, 256)]
    hsp = []
    for b in range(B):
        h = hs[b].T.reshape(NKC, 128, NST, 256).transpose(1, 2, 0, 3)
        hsp.append(np.ascontiguousarray(h.reshape(128, NST * NKC * 256)).astype(NPBF))

    inv_freq = 1.0 / (THETA ** (np.arange(0, D, 2, dtype=np.float32) / D))
    ang = np.outer(inv_freq, np.arange(S, dtype=np.float32))  # (128, S)
    cosp = np.ascontiguousarray(np.cos(ang)).astype(NPBF)
    sinp = np.ascontiguousarray(np.sin(ang)).astype(NPBF)

    invq = np.ascontiguousarray(((1.0 + qnw) ** -2).reshape(2, 128).T).astype(NPBF)
    invk = np.ascontiguousarray(((1.0 + knw) ** -2).reshape(2, 128).T).astype(NPBF)
    onec = np.ones((128, 1), NPBF)
    oner = np.ones((1, 128), NPBF)
    epsb = np.full((1, 1), EPS, np.float32)

    # wo packed: [128, (kc, HID)]
    wop = np.ascontiguousarray(
        wo.reshape(16, 128, HID).transpose(1, 0, 2).reshape(128, 16 * HID)
    ).astype(NPBF)

    qs = 1.0 + qnw
    ks = 1.0 + knw
    in_maps = []
    for core in range(8):
        b, h = core // 4, core % 4
        wq2 = np.concatenate(
            [
                wq[:, h * D : (h + 1) * D] * qs[None, :],
                wq[:, (4 + h) * D : (5 + h) * D] * qs[None, :],
            ],
            axis=1,
        )  # [HID, 512]
        wqp_ = np.stack([pack_w(wq2[:, hc * 128 : (hc + 1) * 128]) for hc in range(4)])
        wk1 = wk[:, h * D : (h + 1) * D] * ks[None, :]
        wkp_ = np.stack([pack_w(wk1[:, hc * 128 : (hc + 1) * 128]) for hc in range(2)])
        wvp_ = pack_w(wv[:, h * D : (h + 1) * D])
        in_maps.append(
            {
                "hsp": hsp[b],
                "wqp": wqp_,
                "wkp": wkp_,
                "wvp": wvp_,
                "wop": wop,
                "cosp": cosp,
                "sinp": sinp,
                "invq": invq,
                "invk": invk,
                "onec": onec,
                "oner": oner,
                "epsb": epsb,
            }
        )
    return in_maps


_PROGRAM = None


def kernel(hidden_states, wq, wk, wv, wo, q_norm_w, k_norm_w):
    global _PROGRAM
    from concourse.bass_utils import run_bass_kernel_spmd

    if _PROGRAM is None:
        _PROGRAM = build_program()
    in_maps = _host_inputs(hidden_states, wq, wk, wv, wo, q_norm_w, k_norm_w)
    res = run_bass_kernel_spmd(_PROGRAM, in_maps, core_ids=list(range(8)))
    out = np.empty((B, S, HID), np.float32)
    for core in range(8):
        b, h = core // 4, core % 4
        out[b, h * 512 : (h + 1) * 512, :] = res.results[core]["out"]
    return out
